# revision 49
# baseline (speedup 1.0000x reference)
"""FFM layer (nn_FFM_Layer) Trainium2 Bass kernel.

Reference computation (B=4096, 13 dense fields, 26 sparse fields with vocab
1000 each, FIELD_NUM=39, K=16):

    idx        = sparse + offsets                      # [B, 26] global ids
    first      = w0 + dense @ w[:13] + sum_j w[idx]    # [B, 1]
    field_f    = einsum('bd,dfk', dense, v[:13]) + sum_j v[idx]   # [B,39,16]
    s          = field_f.sum(1)                        # [B, 16]
    second     = 0.5*(||s||^2 - sum_fk field_f^2)      # [B]
    out        = first + second[:, None]

Strategy (data-parallel over batch, 8 cores x 512 samples, no collectives):
  * Host packs an augmented table V_AUG [26013, 640] f32 (2560 B rows, the
    256B-multiple dma_gather granularity):
      cols [0:624]  = v in K-MAJOR layout (col k*39+f = v[r,f,k])
      col  624      = w[:, 0]  (+ w0 folded into sparse table 0 rows)
      cols [625:640]= V_s[r, 0:15] = per-row field-sums sum_f v[r,f,0:15]
    K-major + V_s pads make the s-vector nearly free: the fold tree and the
    dense matmul sum the pad cols along with everything else, so
    s[0:15] = psum[625:640] and s[15] = one contiguous 39-col reduce.
  * Each core dma_gathers its 512*26 = 13312 rows as ONE flat call stream
    (cols chunk-major: 4 sample-chunks x 26 fields = 104 cols) sized
    [1,1,2,4, 8x9, 4x5, 2,1,1]: small ramp calls because descriptors only
    start executing when a whole call's desc-gen finishes (early doorbells
    start the drain sooner), 8-col calls in the middle for serial Q7
    desc-gen throughput, and a fine-grained back half so the final
    data->fold->stop latency is small.  single_packet=True and queue_num
    round-robin over all 4 SWDGE queues measured fastest.
  * Startup facts (measured): the first DMAGatherAnt cannot execute before
    ~16.5us regardless of inputs (fixed SWDGE/Q7 wake-up after
    LIBRARY_RELOAD; even a zero-input warmup gather waits).  Input DMAs are
    ordered so only the 8-col idx piece gates call 0; the bulk idx pieces,
    ident and the combined [13, 640+512] vaug13+dense tensor are emitted
    after the first gather (DMA-completion waits are cumulative per lane,
    so anything emitted earlier would gate the stream).
  * Per call, DVE folds the gathered cols to one col (pairwise tree;
    tree-on-DVE beats offloading fold levels to PE, whose identity-matmul
    accumulation costs ~1.4us/col with weight reloads vs 0.74us/col adds)
    and PE accumulates it into the chunk's PSUM chain (512/128-col matmul
    split per PSUM bank) seeded by the dense [13,128]x[13,640] matmul,
    emitted 2 calls ahead of each chunk boundary.
  * Epilogue per chunk (emitted 2 calls AFTER the stop matmul so its PSUM
    reads never block the in-order DVE/ACT queues): ACT Square+accum_out
    over ps[0:624] -> sum field_f^2; ACT Identity copy of ps[624:640]
    (w-sum + s[0:15]); DVE 39-col reduce -> s[15]; snorm/diff tiny DVE ops;
    final ACT scale-0.5 + w-sum bias; one output DMA at the end.

Measured on HW (min over reps): 120.5-121.5 us vs 128.8 us baseline.
Breakdown: ~16.5 fixed SWDGE wake-up, ~88-92 desc-gen/drain (all 16 DMA
engines saturated, ~6 ns/row aggregate; Q7 desc-gen ~5-6 ns/row serial is
the pacer), ~10-14 fold/epilogue/output tail.  Host-side prefetch of the
first columns via bulk HWDGE DMA into the wake-up window measured neutral
(the transfer contends with the stream ramp) and was dropped.
"""

import sys

if "/opt/trn_rl_repo" not in sys.path:
    sys.path.insert(0, "/opt/trn_rl_repo")

import numpy as np

import concourse.bacc as bacc
import concourse.bass as bass
import concourse.tile as tile
from concourse import mybir
from concourse.bass_utils import run_bass_kernel_spmd

# Problem constants (hardcoded per harness contract)
B = 4096
N_DENSE = 13
N_SPARSE = 26
FEAT_PER_SPARSE = 1000
FIELD_NUM = 39
FEATURE_NUM = 26013
K = 16
N_CORES = 8
BC = B // N_CORES          # 512 samples per core
ROW = 640                  # 624 v (k-major) + 1 w + 15 V_s  (2560 B)
VCOLS = FIELD_NUM * K      # 624
P = 128
SCHUNKS = BC // P          # 4 sample chunks of 128 per core
TOTCOLS = SCHUNKS * N_SPARSE   # 104 flat gather cols per core

# flat call sizes (cols per dma_gather call).  Small ramp calls make the
# first SWDGE doorbells (and hence the drain) start early -- descriptors
# only begin executing when a whole call's desc-gen completes.  Big calls
# after that maximize serial desc-gen throughput (~5 ns/row + ~1us/call);
# small tail calls shorten the final data/fold latency.
# arranged so round-robin queue_num = k%4 (forced by SWDGE sem-queue
# locking) puts exactly 26 cols on every queue -- the stream end is set
# by the most-loaded queue, and naive ordering loaded queue 0 with 30
CALL_COLS = [1, 2, 1, 4, 8, 8, 8, 8, 8, 8, 8, 8, 4, 4, 8, 4, 4, 4, 1, 2, 1]
assert sum(CALL_COLS) == TOTCOLS
N_QUEUES = 4
# idx tensor split: first piece loads first so call-0 desc-gen isn't gated
# by the full 213KB idx transfer
IDX_SPLITS = [8, 24, TOTCOLS - 32]          # cols per idx sub-tensor
EPI_DEFER = 2                               # calls to defer epilogue emission

F32 = mybir.dt.float32
I16 = mybir.dt.int16


def build_program():
    """Build + compile the single-core SPMD bass program."""
    nc = bacc.Bacc("TRN2", target_bir_lowering=False, debug=False,
                   num_swdge_queues=N_QUEUES)

    vaug_t = nc.dram_tensor("vaug", [FEATURE_NUM, ROW], F32, kind="ExternalInput")
    idxs_t = nc.dram_tensor("idxs", [P, TOTCOLS * 8], I16, kind="ExternalInput")
    combo_t = nc.dram_tensor("combo13", [N_DENSE, ROW + BC], F32,
                             kind="ExternalInput")
    ident_t = nc.dram_tensor("ident", [P, P], F32, kind="ExternalInput")
    out_t = nc.dram_tensor("out", [P, SCHUNKS], F32, kind="ExternalOutput")

    with tile.TileContext(nc) as tc:
        with (
            tc.tile_pool(name="main", bufs=1) as main,
            tc.tile_pool(name="gath", bufs=7) as gath,
            tc.tile_pool(name="f4", bufs=3) as f4p,
            tc.tile_pool(name="f2", bufs=2) as f2p,
            tc.tile_pool(name="f1", bufs=2) as f1p,
            tc.tile_pool(name="sq", bufs=2) as sqp,
            tc.tile_pool(name="small", bufs=2) as small,
            tc.tile_pool(name="psum", bufs=4, space="PSUM") as psum,
        ):
            # idx piece 0 loads FIRST and alone: it is the only DMA gating
            # the ramp gather calls, whose desc-gen should start ~8us.  The
            # remaining input DMAs are emitted AFTER the ramp gathers so no
            # false ordering/semaphore dependency can delay the stream start.
            idx_tiles = []     # (tile, first_col, end_col)
            col0 = 0
            for si, ncols in enumerate(IDX_SPLITS):
                t = main.tile([P, ncols * 8], I16, tag=f"idx{si}")
                idx_tiles.append((t, col0, col0 + ncols))
                col0 += ncols
            nc.sync.dma_start(idx_tiles[0][0][:], idxs_t[:, 0 : IDX_SPLITS[0] * 8])

            def load_idx_piece(si):
                t, lo, hi = idx_tiles[si]
                nc.sync.dma_start(t[:], idxs_t[:, lo * 8 : hi * 8])

            def idx_ap(c0, c1):
                """SBUF idx AP covering gather cols [c0, c1)."""
                for t, lo, hi in idx_tiles:
                    if c0 >= lo and c1 <= hi:
                        return t[:, (c0 - lo) * 8 : (c1 - lo) * 8]
                raise AssertionError((c0, c1))

            ident = main.tile([P, P], F32)
            combo = main.tile([N_DENSE, ROW + BC], F32)
            vaug13 = combo[:, 0:ROW]

            res = main.tile([P, SCHUNKS], F32)

            ps_tiles = {}

            def ensure_chunk(c):
                """Seed chunk c's PSUM chain with the dense matmul."""
                if c in ps_tiles:
                    return ps_tiles[c]
                ps = psum.tile([P, ROW], F32, tag="ps")
                lhs_d = combo[:, ROW + c * P : ROW + (c + 1) * P]
                nc.tensor.matmul(out=ps[:, 0:512], lhsT=lhs_d,
                                 rhs=vaug13[:, 0:512], start=True, stop=False)
                nc.tensor.matmul(out=ps[:, 512:ROW], lhsT=lhs_d,
                                 rhs=vaug13[:, 512:ROW], start=True, stop=False)
                ps_tiles[c] = ps
                return ps

            def accum(ps, cols, stop):
                """PE identity-matmul accumulate [P, ROW] cols into ps."""
                for i, col_ap in enumerate(cols):
                    last = stop and i == len(cols) - 1
                    nc.tensor.matmul(out=ps[:, 0:512], lhsT=ident[:],
                                     rhs=col_ap[:, 0:512], start=False,
                                     stop=last)
                    nc.tensor.matmul(out=ps[:, 512:ROW], lhsT=ident[:],
                                     rhs=col_ap[:, 512:ROW], start=False,
                                     stop=last)

            def fold(g, j0, j1, eng=None):
                """DVE pairwise-tree fold g[:, j0:j1, :] -> one [P, ROW] col.

                Tree-on-DVE beats splitting the reduction with PE mid-stream
                (identity-matmul accumulation costs ~1.4us/col with weight
                reloads vs 0.74us/col DVE adds).  Segment sizes are
                {1,2,3,4,6,8} by construction.
                """
                ve = eng if eng is not None else nc.vector
                n = j1 - j0
                if n == 1:
                    return [g[:, j0, :]]
                if n == 2:
                    t1 = f1p.tile([P, ROW], F32, tag="f1")
                    ve.tensor_add(t1[:], g[:, j0, :], g[:, j0 + 1, :])
                    return [t1]
                if n == 3:
                    t2 = f2p.tile([P, 2, ROW], F32, tag="f2")
                    ve.tensor_add(t2[:, 0:1, :], g[:, j0 : j0 + 1, :],
                                  g[:, j0 + 1 : j0 + 2, :])
                    t1 = f1p.tile([P, ROW], F32, tag="f1")
                    ve.tensor_add(t1[:], t2[:, 0, :], g[:, j0 + 2, :])
                    return [t1]
                if n == 4:
                    t2 = f2p.tile([P, 2, ROW], F32, tag="f2")
                    ve.tensor_add(t2[:], g[:, j0 : j0 + 2, :],
                                  g[:, j0 + 2 : j0 + 4, :])
                    t1 = f1p.tile([P, ROW], F32, tag="f1")
                    ve.tensor_add(t1[:], t2[:, 0, :], t2[:, 1, :])
                    return [t1]
                if n == 6:
                    t4 = f4p.tile([P, 4, ROW], F32, tag="f4")
                    ve.tensor_add(t4[:, 0:3, :], g[:, j0 : j0 + 3, :],
                                  g[:, j0 + 3 : j0 + 6, :])
                    t2 = f2p.tile([P, 2, ROW], F32, tag="f2")
                    ve.tensor_add(t2[:, 0:1, :], t4[:, 0:1, :], t4[:, 1:2, :])
                    t1 = f1p.tile([P, ROW], F32, tag="f1")
                    ve.tensor_add(t1[:], t2[:, 0, :], t4[:, 2, :])
                    return [t1]
                assert n == 8, n
                t4 = f4p.tile([P, 4, ROW], F32, tag="f4")
                ve.tensor_add(t4[:], g[:, j0 : j0 + 4, :],
                              g[:, j0 + 4 : j0 + 8, :])
                t2 = f2p.tile([P, 2, ROW], F32, tag="f2")
                ve.tensor_add(t2[:], t4[:, 0:2, :], t4[:, 2:4, :])
                t1 = f1p.tile([P, ROW], F32, tag="f1")
                ve.tensor_add(t1[:], t2[:, 0, :], t2[:, 1, :])
                return [t1]

            def epilogue(c, ps):
                """FM identity epilogue for chunk c, reading PSUM directly."""
                # q = sum field_f^2  over the 624 v cols
                sqt = sqp.tile([P, VCOLS], F32, tag="sqt")
                q = small.tile([P, 1], F32, tag=f"q{c}")
                nc.scalar.activation(
                    sqt[:], ps[:, 0:VCOLS],
                    mybir.ActivationFunctionType.Square, accum_out=q[:],
                )
                # s[15] = contiguous 39-col reduce of the k=15 block
                st15 = small.tile([P, 1], F32, tag=f"st15{c}")
                nc.vector.tensor_reduce(
                    out=st15[:], in_=ps[:, 15 * FIELD_NUM : VCOLS],
                    op=mybir.AluOpType.add, axis=mybir.AxisListType.X,
                )
                # w-sum + s[0:15] copied out of PSUM once on ACT; the copy
                # also provides the SBUF bias source for the final combine
                wst = small.tile([P, 16], F32, tag=f"wst{c}")
                nc.scalar.activation(
                    wst[:], ps[:, VCOLS:ROW],
                    mybir.ActivationFunctionType.Identity,
                )
                # snorm = sum_{k<15} s_k^2 + s15^2
                s2 = small.tile([P, 15], F32, tag=f"s2{c}")
                sn = small.tile([P, 1], F32, tag=f"sn{c}")
                nc.scalar.activation(
                    s2[:], wst[:, 1:16],
                    mybir.ActivationFunctionType.Square, accum_out=sn[:],
                )
                s15sq = small.tile([P, 1], F32, tag=f"s15sq{c}")
                nc.vector.tensor_tensor(
                    out=s15sq[:], in0=st15[:], in1=st15[:],
                    op=mybir.AluOpType.mult,
                )
                nc.vector.tensor_tensor(
                    out=sn[:], in0=sn[:], in1=s15sq[:],
                    op=mybir.AluOpType.add,
                )
                diff = small.tile([P, 1], F32, tag=f"diff{c}")
                nc.vector.tensor_tensor(
                    out=diff[:], in0=sn[:], in1=q[:],
                    op=mybir.AluOpType.subtract,
                )
                # out = 0.5*diff + (w-sum incl. w0 and dense first-order)
                nc.scalar.activation(
                    res[:, c : c + 1], diff[:],
                    mybir.ActivationFunctionType.Identity,
                    bias=wst[:, 0:1], scale=0.5,
                )

            # precompute per-call col ranges and chunk prepare/epilogue sched
            starts = []
            cum = 0
            for nf in CALL_COLS:
                starts.append((cum, cum + nf))
                cum += nf
            ncalls = len(CALL_COLS)
            first_touch = {}
            for k, (c0, c1) in enumerate(starts):
                for c in range(c0 // N_SPARSE, (c1 - 1) // N_SPARSE + 1):
                    first_touch.setdefault(c, k)
            # prepare chunk c's dense matmuls 2 calls before first touch so
            # PE isn't stalled by them at the boundary
            prep_at = {}
            for c, k in first_touch.items():
                prep_at.setdefault(max(0, k - 2), []).append(c)

            def emit_gather(k):
                c0, c1 = starts[k]
                nf = c1 - c0
                g = gath.tile([P, 8, ROW], F32, tag="g")
                nc.gpsimd.dma_gather(
                    g[:, :nf, :],
                    vaug_t[:],
                    idx_ap(c0, c1),
                    nf * P,
                    nf * P,
                    ROW,
                    single_packet=True,
                    queue_num=k % N_QUEUES,
                )
                return g

            pending_epi = []   # (emit_at_call, chunk, ps)

            def process_call(k, g, eng=None):
                c0, c1 = starts[k]
                nf = c1 - c0
                j = 0
                while j < nf:
                    gcol = c0 + j
                    c = gcol // N_SPARSE
                    seg_end = min(nf, (c + 1) * N_SPARSE - c0)
                    ps = ensure_chunk(c)
                    is_last = (c0 + seg_end) == (c + 1) * N_SPARSE
                    cols = fold(g, j, seg_end, eng=eng)
                    accum(ps, cols, stop=is_last)
                    if is_last:
                        # defer the epilogue so its PSUM reads enter the
                        # in-order DVE/ACT queues after the stop retires
                        pending_epi.append((k + EPI_DEFER, c, ps))
                    j = seg_end
                while pending_epi and pending_epi[0][0] <= k:
                    _, c, ps = pending_epi.pop(0)
                    epilogue(c, ps)

            # call 0's gather first (gated only by idx piece 0), then the
            # remaining input loads, then call 0's processing
            g0 = emit_gather(0)
            load_idx_piece(1)
            load_idx_piece(2)
            nc.sync.dma_start(ident[:], ident_t[:])
            nc.sync.dma_start(combo[:], combo_t[:])
            for c in prep_at.get(0, []):
                ensure_chunk(c)
            process_call(0, g0)

            for k in range(1, ncalls):
                for c in prep_at.get(k, []):
                    ensure_chunk(c)
                g = emit_gather(k)
                process_call(k, g)
            for _, c, ps in pending_epi:
                epilogue(c, ps)

            nc.sync.dma_start(out_t[:], res[:])

    nc.compile()
    return nc


def prep_inputs(dense_inputs, sparse_inputs, w0, w, v):
    """Host-side shard/pack: build per-core in_maps."""
    dense = np.asarray(dense_inputs, np.float32)
    sparse = np.asarray(sparse_inputs)
    w0 = np.asarray(w0, np.float32)
    w = np.asarray(w, np.float32)
    v = np.asarray(v, np.float32)

    vaug = np.zeros((FEATURE_NUM, ROW), np.float32)
    # k-major v layout: col k*39+f = v[r, f, k]
    vaug[:, :VCOLS] = v.transpose(0, 2, 1).reshape(FEATURE_NUM, VCOLS)
    vaug[:, VCOLS] = w[:, 0]
    # fold w0 into sparse table 0 (each sample hits it exactly once)
    vaug[N_DENSE : N_DENSE + FEAT_PER_SPARSE, VCOLS] += w0[0]
    # V_s row field-sums for k=0..14 ride in the pad cols
    vaug[:, VCOLS + 1 : ROW] = v.sum(axis=1)[:, 0:15]

    offs = N_DENSE + FEAT_PER_SPARSE * np.arange(N_SPARSE, dtype=np.int64)
    gidx = (sparse.astype(np.int64) + offs[None, :]).astype(np.int16)  # [B, 26]

    in_maps = []
    for core in range(N_CORES):
        sl = slice(core * BC, (core + 1) * BC)
        idxc = gidx[sl]                                 # [512, 26]
        combo = np.zeros((N_DENSE, ROW + BC), np.float32)
        combo[:, 0:ROW] = vaug[0:N_DENSE]
        combo[:, ROW:] = dense[sl].T                    # [13, 512]

        # flat col order: gcol = chunk*26 + field; call layout per CALL_COLS
        buf = np.zeros((P, TOTCOLS * 8), np.int16)
        cum = 0
        for nf in CALL_COLS:
            n = nf * P
            cols = []
            for gcol in range(cum, cum + nf):
                c, f = divmod(gcol, N_SPARSE)
                cols.append(idxc[c * P : (c + 1) * P, f])   # [128]
            seg = np.stack(cols, 0).reshape(-1)             # [nf*128]
            wrapped = seg.reshape(n // 16, 16).T            # [16, n/16]
            buf[:, cum * 8 : (cum + nf) * 8] = np.tile(wrapped, (8, 1))
            cum += nf
        in_maps.append({"vaug": vaug, "idxs": buf, "combo13": combo,
                        "ident": np.eye(P, dtype=np.float32)})
    return in_maps


_NC_CACHE = None


def kernel(dense_inputs, sparse_inputs, w0, w, v):
    global _NC_CACHE
    if _NC_CACHE is None:
        _NC_CACHE = build_program()
    nc = _NC_CACHE
    in_maps = prep_inputs(dense_inputs, sparse_inputs, w0, w, v)
    res = run_bass_kernel_spmd(nc, in_maps, core_ids=list(range(N_CORES)))
    outs = []
    for r in res.results:
        o = r["out"]                                    # [128, 4]
        outs.append(np.ascontiguousarray(o.T).reshape(BC, 1))
    return np.concatenate(outs, axis=0).astype(np.float32)


# revision 50
# speedup vs baseline: 1.0401x; 1.0401x over previous
"""FFM layer (nn_FFM_Layer) Trainium2 Bass kernel.

Reference computation (B=4096, 13 dense fields, 26 sparse fields with vocab
1000 each, FIELD_NUM=39, K=16):

    idx        = sparse + offsets                      # [B, 26] global ids
    first      = w0 + dense @ w[:13] + sum_j w[idx]    # [B, 1]
    field_f    = einsum('bd,dfk', dense, v[:13]) + sum_j v[idx]   # [B,39,16]
    s          = field_f.sum(1)                        # [B, 16]
    second     = 0.5*(||s||^2 - sum_fk field_f^2)      # [B]
    out        = first + second[:, None]

Strategy (data-parallel over batch, 8 cores x 512 samples, no collectives):
  * Host packs an augmented table V_AUG [26013, 640] f32 (2560 B rows, the
    256B-multiple dma_gather granularity):
      cols [0:624]  = v in K-MAJOR layout (col k*39+f = v[r,f,k])
      col  624      = w[:, 0]  (+ w0 folded into sparse table 0 rows)
      cols [625:640]= V_s[r, 0:15] = per-row field-sums sum_f v[r,f,0:15]
    K-major + V_s pads make the s-vector nearly free: the fold tree and the
    dense matmul sum the pad cols along with everything else, so
    s[0:15] = psum[625:640] and s[15] = one contiguous 39-col reduce.
  * Each core dma_gathers its 512*26 = 13312 rows as ONE flat call stream
    (cols chunk-major: 4 sample-chunks x 26 fields = 104 cols) sized
    [1,1,2,4, 8x9, 4x5, 2,1,1]: small ramp calls because descriptors only
    start executing when a whole call's desc-gen finishes (early doorbells
    start the drain sooner), 8-col calls in the middle for serial Q7
    desc-gen throughput, and a fine-grained back half so the final
    data->fold->stop latency is small.  single_packet=True and queue_num
    round-robin over all 4 SWDGE queues measured fastest.
  * Startup facts (measured): the first DMAGatherAnt cannot execute before
    ~16.5us regardless of inputs (fixed SWDGE/Q7 wake-up after
    LIBRARY_RELOAD; even a zero-input warmup gather waits).  Input DMAs are
    ordered so only the 8-col idx piece gates call 0; the bulk idx pieces,
    ident and the combined [13, 640+512] vaug13+dense tensor are emitted
    after the first gather (DMA-completion waits are cumulative per lane,
    so anything emitted earlier would gate the stream).
  * Per call, DVE folds the gathered cols to one col (pairwise tree;
    tree-on-DVE beats offloading fold levels to PE, whose identity-matmul
    accumulation costs ~1.4us/col with weight reloads vs 0.74us/col adds)
    and PE accumulates it into the chunk's PSUM chain (512/128-col matmul
    split per PSUM bank) seeded by the dense [13,128]x[13,640] matmul,
    emitted 2 calls ahead of each chunk boundary.
  * Epilogue per chunk (emitted 2 calls AFTER the stop matmul so its PSUM
    reads never block the in-order DVE/ACT queues): ACT Square+accum_out
    over ps[0:624] -> sum field_f^2; ACT Identity copy of ps[624:640]
    (w-sum + s[0:15]); DVE 39-col reduce -> s[15]; snorm/diff tiny DVE ops;
    final ACT scale-0.5 + w-sum bias; one output DMA at the end.

Measured on HW (min over reps): 120.5-121.5 us vs 128.8 us baseline.
Breakdown: ~16.5 fixed SWDGE wake-up, ~88-92 desc-gen/drain (all 16 DMA
engines saturated, ~6 ns/row aggregate; Q7 desc-gen ~5-6 ns/row serial is
the pacer), ~10-14 fold/epilogue/output tail.  Host-side prefetch of the
first columns via bulk HWDGE DMA into the wake-up window measured neutral
(the transfer contends with the stream ramp) and was dropped.
"""

import sys

if "/opt/trn_rl_repo" not in sys.path:
    sys.path.insert(0, "/opt/trn_rl_repo")

import numpy as np

import concourse.bacc as bacc
import concourse.bass as bass
import concourse.tile as tile
from concourse import mybir
from concourse.bass_utils import run_bass_kernel_spmd

# Problem constants (hardcoded per harness contract)
B = 4096
N_DENSE = 13
N_SPARSE = 26
FEAT_PER_SPARSE = 1000
FIELD_NUM = 39
FEATURE_NUM = 26013
K = 16
N_CORES = 8
BC = B // N_CORES          # 512 samples per core
ROW = 640                  # 624 v (k-major) + 1 w + 15 V_s  (2560 B)
VCOLS = FIELD_NUM * K      # 624
P = 128
SCHUNKS = BC // P          # 4 sample chunks of 128 per core
TOTCOLS = SCHUNKS * N_SPARSE   # 104 flat gather cols per core

# flat call sizes (cols per dma_gather call).  Small ramp calls make the
# first SWDGE doorbells (and hence the drain) start early -- descriptors
# only begin executing when a whole call's desc-gen completes.  Big calls
# after that maximize serial desc-gen throughput (~5 ns/row + ~1us/call);
# small tail calls shorten the final data/fold latency.
CALL_COLS = [1, 1, 2, 4] + [8] * 9 + [4] * 5 + [2, 1, 1]
assert sum(CALL_COLS) == TOTCOLS
N_QUEUES = 4
# idx tensor split: first piece loads first so call-0 desc-gen isn't gated
# by the full 213KB idx transfer
IDX_SPLITS = [8, 24, TOTCOLS - 32]          # cols per idx sub-tensor
EPI_DEFER = 2                               # calls to defer epilogue emission

F32 = mybir.dt.float32
I16 = mybir.dt.int16


def build_program():
    """Build + compile the single-core SPMD bass program."""
    nc = bacc.Bacc("TRN2", target_bir_lowering=False, debug=False,
                   num_swdge_queues=N_QUEUES)

    vaug_t = nc.dram_tensor("vaug", [FEATURE_NUM, ROW], F32, kind="ExternalInput")
    idxs_t = nc.dram_tensor("idxs", [P, TOTCOLS * 8], I16, kind="ExternalInput")
    combo_t = nc.dram_tensor("combo13", [N_DENSE, ROW + BC], F32,
                             kind="ExternalInput")
    ident_t = nc.dram_tensor("ident", [P, P], F32, kind="ExternalInput")
    out_t = nc.dram_tensor("out", [P, SCHUNKS], F32, kind="ExternalOutput")

    with tile.TileContext(nc) as tc:
        with (
            tc.tile_pool(name="main", bufs=1) as main,
            tc.tile_pool(name="gath", bufs=7) as gath,
            tc.tile_pool(name="f4", bufs=3) as f4p,
            tc.tile_pool(name="f2", bufs=2) as f2p,
            tc.tile_pool(name="f1", bufs=2) as f1p,
            tc.tile_pool(name="sq", bufs=2) as sqp,
            tc.tile_pool(name="small", bufs=2) as small,
            tc.tile_pool(name="psum", bufs=4, space="PSUM") as psum,
        ):
            # idx piece 0 loads FIRST and alone: it is the only DMA gating
            # the ramp gather calls, whose desc-gen should start ~8us.  The
            # remaining input DMAs are emitted AFTER the ramp gathers so no
            # false ordering/semaphore dependency can delay the stream start.
            idx_tiles = []     # (tile, first_col, end_col)
            col0 = 0
            for si, ncols in enumerate(IDX_SPLITS):
                t = main.tile([P, ncols * 8], I16, tag=f"idx{si}")
                idx_tiles.append((t, col0, col0 + ncols))
                col0 += ncols
            nc.sync.dma_start(idx_tiles[0][0][:], idxs_t[:, 0 : IDX_SPLITS[0] * 8])

            def load_idx_piece(si):
                t, lo, hi = idx_tiles[si]
                nc.sync.dma_start(t[:], idxs_t[:, lo * 8 : hi * 8])

            def idx_ap(c0, c1):
                """SBUF idx AP covering gather cols [c0, c1)."""
                for t, lo, hi in idx_tiles:
                    if c0 >= lo and c1 <= hi:
                        return t[:, (c0 - lo) * 8 : (c1 - lo) * 8]
                raise AssertionError((c0, c1))

            ident = main.tile([P, P], F32)
            combo = main.tile([N_DENSE, ROW + BC], F32)
            vaug13 = combo[:, 0:ROW]

            res = main.tile([P, SCHUNKS], F32)

            ps_tiles = {}

            def ensure_chunk(c):
                """Seed chunk c's PSUM chain with the dense matmul."""
                if c in ps_tiles:
                    return ps_tiles[c]
                ps = psum.tile([P, ROW], F32, tag="ps")
                lhs_d = combo[:, ROW + c * P : ROW + (c + 1) * P]
                nc.tensor.matmul(out=ps[:, 0:512], lhsT=lhs_d,
                                 rhs=vaug13[:, 0:512], start=True, stop=False)
                nc.tensor.matmul(out=ps[:, 512:ROW], lhsT=lhs_d,
                                 rhs=vaug13[:, 512:ROW], start=True, stop=False)
                ps_tiles[c] = ps
                return ps

            def accum(ps, cols, stop):
                """PE identity-matmul accumulate [P, ROW] cols into ps."""
                for i, col_ap in enumerate(cols):
                    last = stop and i == len(cols) - 1
                    nc.tensor.matmul(out=ps[:, 0:512], lhsT=ident[:],
                                     rhs=col_ap[:, 0:512], start=False,
                                     stop=last)
                    nc.tensor.matmul(out=ps[:, 512:ROW], lhsT=ident[:],
                                     rhs=col_ap[:, 512:ROW], start=False,
                                     stop=last)

            def fold(g, j0, j1, eng=None):
                """DVE pairwise-tree fold g[:, j0:j1, :] -> one [P, ROW] col.

                Tree-on-DVE beats splitting the reduction with PE mid-stream
                (identity-matmul accumulation costs ~1.4us/col with weight
                reloads vs 0.74us/col DVE adds).  Segment sizes are
                {1,2,3,4,6,8} by construction.
                """
                ve = eng if eng is not None else nc.vector
                n = j1 - j0
                if n == 1:
                    return [g[:, j0, :]]
                if n == 2:
                    t1 = f1p.tile([P, ROW], F32, tag="f1")
                    ve.tensor_add(t1[:], g[:, j0, :], g[:, j0 + 1, :])
                    return [t1]
                if n == 3:
                    t2 = f2p.tile([P, 2, ROW], F32, tag="f2")
                    ve.tensor_add(t2[:, 0:1, :], g[:, j0 : j0 + 1, :],
                                  g[:, j0 + 1 : j0 + 2, :])
                    t1 = f1p.tile([P, ROW], F32, tag="f1")
                    ve.tensor_add(t1[:], t2[:, 0, :], g[:, j0 + 2, :])
                    return [t1]
                if n == 4:
                    t2 = f2p.tile([P, 2, ROW], F32, tag="f2")
                    ve.tensor_add(t2[:], g[:, j0 : j0 + 2, :],
                                  g[:, j0 + 2 : j0 + 4, :])
                    t1 = f1p.tile([P, ROW], F32, tag="f1")
                    ve.tensor_add(t1[:], t2[:, 0, :], t2[:, 1, :])
                    return [t1]
                if n == 6:
                    t4 = f4p.tile([P, 4, ROW], F32, tag="f4")
                    ve.tensor_add(t4[:, 0:3, :], g[:, j0 : j0 + 3, :],
                                  g[:, j0 + 3 : j0 + 6, :])
                    t2 = f2p.tile([P, 2, ROW], F32, tag="f2")
                    ve.tensor_add(t2[:, 0:1, :], t4[:, 0:1, :], t4[:, 1:2, :])
                    t1 = f1p.tile([P, ROW], F32, tag="f1")
                    ve.tensor_add(t1[:], t2[:, 0, :], t4[:, 2, :])
                    return [t1]
                assert n == 8, n
                t4 = f4p.tile([P, 4, ROW], F32, tag="f4")
                ve.tensor_add(t4[:], g[:, j0 : j0 + 4, :],
                              g[:, j0 + 4 : j0 + 8, :])
                t2 = f2p.tile([P, 2, ROW], F32, tag="f2")
                ve.tensor_add(t2[:], t4[:, 0:2, :], t4[:, 2:4, :])
                t1 = f1p.tile([P, ROW], F32, tag="f1")
                ve.tensor_add(t1[:], t2[:, 0, :], t2[:, 1, :])
                return [t1]

            def epilogue(c, ps):
                """FM identity epilogue for chunk c, reading PSUM directly."""
                # q = sum field_f^2  over the 624 v cols
                sqt = sqp.tile([P, VCOLS], F32, tag="sqt")
                q = small.tile([P, 1], F32, tag=f"q{c}")
                nc.scalar.activation(
                    sqt[:], ps[:, 0:VCOLS],
                    mybir.ActivationFunctionType.Square, accum_out=q[:],
                )
                # s[15] = contiguous 39-col reduce of the k=15 block
                st15 = small.tile([P, 1], F32, tag=f"st15{c}")
                nc.vector.tensor_reduce(
                    out=st15[:], in_=ps[:, 15 * FIELD_NUM : VCOLS],
                    op=mybir.AluOpType.add, axis=mybir.AxisListType.X,
                )
                # w-sum + s[0:15] copied out of PSUM once on ACT; the copy
                # also provides the SBUF bias source for the final combine
                wst = small.tile([P, 16], F32, tag=f"wst{c}")
                nc.scalar.activation(
                    wst[:], ps[:, VCOLS:ROW],
                    mybir.ActivationFunctionType.Identity,
                )
                # snorm = sum_{k<15} s_k^2 + s15^2
                s2 = small.tile([P, 15], F32, tag=f"s2{c}")
                sn = small.tile([P, 1], F32, tag=f"sn{c}")
                nc.scalar.activation(
                    s2[:], wst[:, 1:16],
                    mybir.ActivationFunctionType.Square, accum_out=sn[:],
                )
                s15sq = small.tile([P, 1], F32, tag=f"s15sq{c}")
                nc.vector.tensor_tensor(
                    out=s15sq[:], in0=st15[:], in1=st15[:],
                    op=mybir.AluOpType.mult,
                )
                nc.vector.tensor_tensor(
                    out=sn[:], in0=sn[:], in1=s15sq[:],
                    op=mybir.AluOpType.add,
                )
                diff = small.tile([P, 1], F32, tag=f"diff{c}")
                nc.vector.tensor_tensor(
                    out=diff[:], in0=sn[:], in1=q[:],
                    op=mybir.AluOpType.subtract,
                )
                # out = 0.5*diff + (w-sum incl. w0 and dense first-order)
                nc.scalar.activation(
                    res[:, c : c + 1], diff[:],
                    mybir.ActivationFunctionType.Identity,
                    bias=wst[:, 0:1], scale=0.5,
                )

            # precompute per-call col ranges and chunk prepare/epilogue sched
            starts = []
            cum = 0
            for nf in CALL_COLS:
                starts.append((cum, cum + nf))
                cum += nf
            ncalls = len(CALL_COLS)
            first_touch = {}
            for k, (c0, c1) in enumerate(starts):
                for c in range(c0 // N_SPARSE, (c1 - 1) // N_SPARSE + 1):
                    first_touch.setdefault(c, k)
            # prepare chunk c's dense matmuls 2 calls before first touch so
            # PE isn't stalled by them at the boundary
            prep_at = {}
            for c, k in first_touch.items():
                prep_at.setdefault(max(0, k - 2), []).append(c)

            def emit_gather(k):
                c0, c1 = starts[k]
                nf = c1 - c0
                g = gath.tile([P, 8, ROW], F32, tag="g")
                nc.gpsimd.dma_gather(
                    g[:, :nf, :],
                    vaug_t[:],
                    idx_ap(c0, c1),
                    nf * P,
                    nf * P,
                    ROW,
                    single_packet=True,
                    queue_num=k % N_QUEUES,
                )
                return g

            pending_epi = []   # (emit_at_call, chunk, ps)

            def process_call(k, g, eng=None):
                c0, c1 = starts[k]
                nf = c1 - c0
                j = 0
                while j < nf:
                    gcol = c0 + j
                    c = gcol // N_SPARSE
                    seg_end = min(nf, (c + 1) * N_SPARSE - c0)
                    ps = ensure_chunk(c)
                    is_last = (c0 + seg_end) == (c + 1) * N_SPARSE
                    cols = fold(g, j, seg_end, eng=eng)
                    accum(ps, cols, stop=is_last)
                    if is_last:
                        # defer the epilogue so its PSUM reads enter the
                        # in-order DVE/ACT queues after the stop retires
                        pending_epi.append((k + EPI_DEFER, c, ps))
                    j = seg_end
                while pending_epi and pending_epi[0][0] <= k:
                    _, c, ps = pending_epi.pop(0)
                    epilogue(c, ps)

            # call 0's gather first (gated only by idx piece 0), then the
            # remaining input loads, then call 0's processing
            g0 = emit_gather(0)
            load_idx_piece(1)
            load_idx_piece(2)
            nc.sync.dma_start(ident[:], ident_t[:])
            nc.sync.dma_start(combo[:], combo_t[:])
            for c in prep_at.get(0, []):
                ensure_chunk(c)
            process_call(0, g0)

            for k in range(1, ncalls):
                for c in prep_at.get(k, []):
                    ensure_chunk(c)
                g = emit_gather(k)
                process_call(k, g)
            for _, c, ps in pending_epi:
                epilogue(c, ps)

            nc.sync.dma_start(out_t[:], res[:])

    nc.compile()
    return nc


def prep_inputs(dense_inputs, sparse_inputs, w0, w, v):
    """Host-side shard/pack: build per-core in_maps."""
    dense = np.asarray(dense_inputs, np.float32)
    sparse = np.asarray(sparse_inputs)
    w0 = np.asarray(w0, np.float32)
    w = np.asarray(w, np.float32)
    v = np.asarray(v, np.float32)

    vaug = np.zeros((FEATURE_NUM, ROW), np.float32)
    # k-major v layout: col k*39+f = v[r, f, k]
    vaug[:, :VCOLS] = v.transpose(0, 2, 1).reshape(FEATURE_NUM, VCOLS)
    vaug[:, VCOLS] = w[:, 0]
    # fold w0 into sparse table 0 (each sample hits it exactly once)
    vaug[N_DENSE : N_DENSE + FEAT_PER_SPARSE, VCOLS] += w0[0]
    # V_s row field-sums for k=0..14 ride in the pad cols
    vaug[:, VCOLS + 1 : ROW] = v.sum(axis=1)[:, 0:15]

    offs = N_DENSE + FEAT_PER_SPARSE * np.arange(N_SPARSE, dtype=np.int64)
    gidx = (sparse.astype(np.int64) + offs[None, :]).astype(np.int16)  # [B, 26]

    in_maps = []
    for core in range(N_CORES):
        sl = slice(core * BC, (core + 1) * BC)
        idxc = gidx[sl]                                 # [512, 26]
        combo = np.zeros((N_DENSE, ROW + BC), np.float32)
        combo[:, 0:ROW] = vaug[0:N_DENSE]
        combo[:, ROW:] = dense[sl].T                    # [13, 512]

        # flat col order: gcol = chunk*26 + field; call layout per CALL_COLS
        buf = np.zeros((P, TOTCOLS * 8), np.int16)
        cum = 0
        for nf in CALL_COLS:
            n = nf * P
            cols = []
            for gcol in range(cum, cum + nf):
                c, f = divmod(gcol, N_SPARSE)
                cols.append(idxc[c * P : (c + 1) * P, f])   # [128]
            seg = np.stack(cols, 0).reshape(-1)             # [nf*128]
            wrapped = seg.reshape(n // 16, 16).T            # [16, n/16]
            buf[:, cum * 8 : (cum + nf) * 8] = np.tile(wrapped, (8, 1))
            cum += nf
        in_maps.append({"vaug": vaug, "idxs": buf, "combo13": combo,
                        "ident": np.eye(P, dtype=np.float32)})
    return in_maps


_NC_CACHE = None


def kernel(dense_inputs, sparse_inputs, w0, w, v):
    global _NC_CACHE
    if _NC_CACHE is None:
        _NC_CACHE = build_program()
    nc = _NC_CACHE
    in_maps = prep_inputs(dense_inputs, sparse_inputs, w0, w, v)
    res = run_bass_kernel_spmd(nc, in_maps, core_ids=list(range(N_CORES)))
    outs = []
    for r in res.results:
        o = r["out"]                                    # [128, 4]
        outs.append(np.ascontiguousarray(o.T).reshape(BC, 1))
    return np.concatenate(outs, axis=0).astype(np.float32)


# revision 52
# speedup vs baseline: 1.0432x; 1.0030x over previous
"""FFM layer (nn_FFM_Layer) Trainium2 Bass kernel.

Reference computation (B=4096, 13 dense fields, 26 sparse fields with vocab
1000 each, FIELD_NUM=39, K=16):

    idx        = sparse + offsets                      # [B, 26] global ids
    first      = w0 + dense @ w[:13] + sum_j w[idx]    # [B, 1]
    field_f    = einsum('bd,dfk', dense, v[:13]) + sum_j v[idx]   # [B,39,16]
    s          = field_f.sum(1)                        # [B, 16]
    second     = 0.5*(||s||^2 - sum_fk field_f^2)      # [B]
    out        = first + second[:, None]

Strategy (data-parallel over batch, 8 cores x 512 samples, no collectives):
  * Host packs an augmented table V_AUG [26013, 640] f32 (2560 B rows, the
    256B-multiple dma_gather granularity):
      cols [0:624]  = v in K-MAJOR layout (col k*39+f = v[r,f,k])
      col  624      = w[:, 0]  (+ w0 folded into sparse table 0 rows)
      cols [625:640]= V_s[r, 0:15] = per-row field-sums sum_f v[r,f,0:15]
    K-major + V_s pads make the s-vector nearly free: the fold tree and the
    dense matmul sum the pad cols along with everything else, so
    s[0:15] = psum[625:640] and s[15] = one contiguous 39-col reduce.
  * Each core dma_gathers its 512*26 = 13312 rows as ONE flat call stream
    (cols chunk-major: 4 sample-chunks x 26 fields = 104 cols) sized
    [1,1,2,4, 8x9, 4x5, 2,1,1]: small ramp calls because descriptors only
    start executing when a whole call's desc-gen finishes (early doorbells
    start the drain sooner), 8-col calls in the middle for serial Q7
    desc-gen throughput, and a fine-grained back half so the final
    data->fold->stop latency is small.  single_packet=True and queue_num
    round-robin over all 4 SWDGE queues measured fastest.
  * Startup facts (measured): the first DMAGatherAnt cannot execute before
    ~16.5us regardless of inputs (fixed SWDGE/Q7 wake-up after
    LIBRARY_RELOAD; even a zero-input warmup gather waits).  Input DMAs are
    ordered so only the 8-col idx piece gates call 0; the bulk idx pieces,
    ident and the combined [13, 640+512] vaug13+dense tensor are emitted
    after the first gather (DMA-completion waits are cumulative per lane,
    so anything emitted earlier would gate the stream).
  * Per call, DVE folds the gathered cols to one col (pairwise tree;
    tree-on-DVE beats offloading fold levels to PE, whose identity-matmul
    accumulation costs ~1.4us/col with weight reloads vs 0.74us/col adds)
    and PE accumulates it into the chunk's PSUM chain (512/128-col matmul
    split per PSUM bank) seeded by the dense [13,128]x[13,640] matmul,
    emitted 2 calls ahead of each chunk boundary.
  * Epilogue per chunk (emitted 2 calls AFTER the stop matmul so its PSUM
    reads never block the in-order DVE/ACT queues): ACT Square+accum_out
    over ps[0:624] -> sum field_f^2; ACT Identity copy of ps[624:640]
    (w-sum + s[0:15]); DVE 39-col reduce -> s[15]; snorm/diff tiny DVE ops;
    final ACT scale-0.5 + w-sum bias; one output DMA at the end.

Measured on HW (min over reps): 120.5-121.5 us vs 128.8 us baseline.
Breakdown: ~16.5 fixed SWDGE wake-up, ~88-92 desc-gen/drain (all 16 DMA
engines saturated, ~6 ns/row aggregate; Q7 desc-gen ~5-6 ns/row serial is
the pacer), ~10-14 fold/epilogue/output tail.  Host-side prefetch of the
first columns via bulk HWDGE DMA into the wake-up window measured neutral
(the transfer contends with the stream ramp) and was dropped.
"""

import sys

if "/opt/trn_rl_repo" not in sys.path:
    sys.path.insert(0, "/opt/trn_rl_repo")

import numpy as np

import concourse.bacc as bacc
import concourse.bass as bass
import concourse.tile as tile
from concourse import mybir
from concourse.bass_utils import run_bass_kernel_spmd

# Problem constants (hardcoded per harness contract)
B = 4096
N_DENSE = 13
N_SPARSE = 26
FEAT_PER_SPARSE = 1000
FIELD_NUM = 39
FEATURE_NUM = 26013
K = 16
N_CORES = 8
BC = B // N_CORES          # 512 samples per core
ROW = 640                  # 624 v (k-major) + 1 w + 15 V_s  (2560 B)
VCOLS = FIELD_NUM * K      # 624
P = 128
SCHUNKS = BC // P          # 4 sample chunks of 128 per core
TOTCOLS = SCHUNKS * N_SPARSE   # 104 flat gather cols per core

# flat call sizes (cols per dma_gather call).  Small ramp calls make the
# first SWDGE doorbells (and hence the drain) start early -- descriptors
# only begin executing when a whole call's desc-gen completes.  Big calls
# after that maximize serial desc-gen throughput (~5 ns/row + ~1us/call);
# small tail calls shorten the final data/fold latency.
# chunk-aligned: no call straddles a 26-col chunk boundary, so no call's
# gather buffer is held across two folds + a stop matmul (the straddle
# backpressure showed as periodic drain dips every ~20us in the trace)
CALL_COLS = [1, 1, 2, 4, 8, 8, 2] + [8, 8, 8, 2] + [8, 8, 8, 2] + [8, 8, 4, 4, 2]
assert sum(CALL_COLS) == TOTCOLS
N_QUEUES = 4
# idx tensor split: first piece loads first so call-0 desc-gen isn't gated
# by the full 213KB idx transfer
IDX_SPLITS = [8, 18, TOTCOLS - 26]          # cols per idx sub-tensor
EPI_DEFER = 2                               # calls to defer epilogue emission

F32 = mybir.dt.float32
I16 = mybir.dt.int16


def build_program():
    """Build + compile the single-core SPMD bass program."""
    nc = bacc.Bacc("TRN2", target_bir_lowering=False, debug=False,
                   num_swdge_queues=N_QUEUES)

    vaug_t = nc.dram_tensor("vaug", [FEATURE_NUM, ROW], F32, kind="ExternalInput")
    idxs_t = nc.dram_tensor("idxs", [P, TOTCOLS * 8], I16, kind="ExternalInput")
    combo_t = nc.dram_tensor("combo13", [N_DENSE, ROW + BC], F32,
                             kind="ExternalInput")
    ident_t = nc.dram_tensor("ident", [P, P], F32, kind="ExternalInput")
    out_t = nc.dram_tensor("out", [P, SCHUNKS], F32, kind="ExternalOutput")

    with tile.TileContext(nc) as tc:
        with (
            tc.tile_pool(name="main", bufs=1) as main,
            tc.tile_pool(name="gath", bufs=7) as gath,
            tc.tile_pool(name="f4", bufs=3) as f4p,
            tc.tile_pool(name="f2", bufs=2) as f2p,
            tc.tile_pool(name="f1", bufs=2) as f1p,
            tc.tile_pool(name="sq", bufs=2) as sqp,
            tc.tile_pool(name="small", bufs=2) as small,
            tc.tile_pool(name="psum", bufs=4, space="PSUM") as psum,
        ):
            # idx piece 0 loads FIRST and alone: it is the only DMA gating
            # the ramp gather calls, whose desc-gen should start ~8us.  The
            # remaining input DMAs are emitted AFTER the ramp gathers so no
            # false ordering/semaphore dependency can delay the stream start.
            idx_tiles = []     # (tile, first_col, end_col)
            col0 = 0
            for si, ncols in enumerate(IDX_SPLITS):
                t = main.tile([P, ncols * 8], I16, tag=f"idx{si}")
                idx_tiles.append((t, col0, col0 + ncols))
                col0 += ncols
            nc.sync.dma_start(idx_tiles[0][0][:], idxs_t[:, 0 : IDX_SPLITS[0] * 8])

            def load_idx_piece(si):
                t, lo, hi = idx_tiles[si]
                nc.sync.dma_start(t[:], idxs_t[:, lo * 8 : hi * 8])

            def idx_ap(c0, c1):
                """SBUF idx AP covering gather cols [c0, c1)."""
                for t, lo, hi in idx_tiles:
                    if c0 >= lo and c1 <= hi:
                        return t[:, (c0 - lo) * 8 : (c1 - lo) * 8]
                raise AssertionError((c0, c1))

            ident = main.tile([P, P], F32)
            combo = main.tile([N_DENSE, ROW + BC], F32)
            vaug13 = combo[:, 0:ROW]

            res = main.tile([P, SCHUNKS], F32)

            ps_tiles = {}

            def ensure_chunk(c):
                """Seed chunk c's PSUM chain with the dense matmul."""
                if c in ps_tiles:
                    return ps_tiles[c]
                ps = psum.tile([P, ROW], F32, tag="ps")
                lhs_d = combo[:, ROW + c * P : ROW + (c + 1) * P]
                nc.tensor.matmul(out=ps[:, 0:512], lhsT=lhs_d,
                                 rhs=vaug13[:, 0:512], start=True, stop=False)
                nc.tensor.matmul(out=ps[:, 512:ROW], lhsT=lhs_d,
                                 rhs=vaug13[:, 512:ROW], start=True, stop=False)
                ps_tiles[c] = ps
                return ps

            def accum(ps, cols, stop):
                """PE identity-matmul accumulate [P, ROW] cols into ps."""
                for i, col_ap in enumerate(cols):
                    last = stop and i == len(cols) - 1
                    nc.tensor.matmul(out=ps[:, 0:512], lhsT=ident[:],
                                     rhs=col_ap[:, 0:512], start=False,
                                     stop=last)
                    nc.tensor.matmul(out=ps[:, 512:ROW], lhsT=ident[:],
                                     rhs=col_ap[:, 512:ROW], start=False,
                                     stop=last)

            def fold(g, j0, j1, eng=None):
                """DVE pairwise-tree fold g[:, j0:j1, :] -> one [P, ROW] col.

                Tree-on-DVE beats splitting the reduction with PE mid-stream
                (identity-matmul accumulation costs ~1.4us/col with weight
                reloads vs 0.74us/col DVE adds).  Segment sizes are
                {1,2,3,4,6,8} by construction.
                """
                ve = eng if eng is not None else nc.vector
                n = j1 - j0
                if n == 1:
                    return [g[:, j0, :]]
                if n == 2:
                    t1 = f1p.tile([P, ROW], F32, tag="f1")
                    ve.tensor_add(t1[:], g[:, j0, :], g[:, j0 + 1, :])
                    return [t1]
                if n == 3:
                    t2 = f2p.tile([P, 2, ROW], F32, tag="f2")
                    ve.tensor_add(t2[:, 0:1, :], g[:, j0 : j0 + 1, :],
                                  g[:, j0 + 1 : j0 + 2, :])
                    t1 = f1p.tile([P, ROW], F32, tag="f1")
                    ve.tensor_add(t1[:], t2[:, 0, :], g[:, j0 + 2, :])
                    return [t1]
                if n == 4:
                    t2 = f2p.tile([P, 2, ROW], F32, tag="f2")
                    ve.tensor_add(t2[:], g[:, j0 : j0 + 2, :],
                                  g[:, j0 + 2 : j0 + 4, :])
                    t1 = f1p.tile([P, ROW], F32, tag="f1")
                    ve.tensor_add(t1[:], t2[:, 0, :], t2[:, 1, :])
                    return [t1]
                if n == 6:
                    t4 = f4p.tile([P, 4, ROW], F32, tag="f4")
                    ve.tensor_add(t4[:, 0:3, :], g[:, j0 : j0 + 3, :],
                                  g[:, j0 + 3 : j0 + 6, :])
                    t2 = f2p.tile([P, 2, ROW], F32, tag="f2")
                    ve.tensor_add(t2[:, 0:1, :], t4[:, 0:1, :], t4[:, 1:2, :])
                    t1 = f1p.tile([P, ROW], F32, tag="f1")
                    ve.tensor_add(t1[:], t2[:, 0, :], t4[:, 2, :])
                    return [t1]
                assert n == 8, n
                t4 = f4p.tile([P, 4, ROW], F32, tag="f4")
                ve.tensor_add(t4[:], g[:, j0 : j0 + 4, :],
                              g[:, j0 + 4 : j0 + 8, :])
                t2 = f2p.tile([P, 2, ROW], F32, tag="f2")
                ve.tensor_add(t2[:], t4[:, 0:2, :], t4[:, 2:4, :])
                t1 = f1p.tile([P, ROW], F32, tag="f1")
                ve.tensor_add(t1[:], t2[:, 0, :], t2[:, 1, :])
                return [t1]

            def epilogue(c, ps):
                """FM identity epilogue for chunk c, reading PSUM directly."""
                # q = sum field_f^2  over the 624 v cols
                sqt = sqp.tile([P, VCOLS], F32, tag="sqt")
                q = small.tile([P, 1], F32, tag=f"q{c}")
                nc.scalar.activation(
                    sqt[:], ps[:, 0:VCOLS],
                    mybir.ActivationFunctionType.Square, accum_out=q[:],
                )
                # s[15] = contiguous 39-col reduce of the k=15 block
                st15 = small.tile([P, 1], F32, tag=f"st15{c}")
                nc.vector.tensor_reduce(
                    out=st15[:], in_=ps[:, 15 * FIELD_NUM : VCOLS],
                    op=mybir.AluOpType.add, axis=mybir.AxisListType.X,
                )
                # w-sum + s[0:15] copied out of PSUM once on ACT; the copy
                # also provides the SBUF bias source for the final combine
                wst = small.tile([P, 16], F32, tag=f"wst{c}")
                nc.scalar.activation(
                    wst[:], ps[:, VCOLS:ROW],
                    mybir.ActivationFunctionType.Identity,
                )
                # snorm = sum_{k<15} s_k^2 + s15^2
                s2 = small.tile([P, 15], F32, tag=f"s2{c}")
                sn = small.tile([P, 1], F32, tag=f"sn{c}")
                nc.scalar.activation(
                    s2[:], wst[:, 1:16],
                    mybir.ActivationFunctionType.Square, accum_out=sn[:],
                )
                s15sq = small.tile([P, 1], F32, tag=f"s15sq{c}")
                nc.vector.tensor_tensor(
                    out=s15sq[:], in0=st15[:], in1=st15[:],
                    op=mybir.AluOpType.mult,
                )
                nc.vector.tensor_tensor(
                    out=sn[:], in0=sn[:], in1=s15sq[:],
                    op=mybir.AluOpType.add,
                )
                diff = small.tile([P, 1], F32, tag=f"diff{c}")
                nc.vector.tensor_tensor(
                    out=diff[:], in0=sn[:], in1=q[:],
                    op=mybir.AluOpType.subtract,
                )
                # out = 0.5*diff + (w-sum incl. w0 and dense first-order)
                nc.scalar.activation(
                    res[:, c : c + 1], diff[:],
                    mybir.ActivationFunctionType.Identity,
                    bias=wst[:, 0:1], scale=0.5,
                )

            # precompute per-call col ranges and chunk prepare/epilogue sched
            starts = []
            cum = 0
            for nf in CALL_COLS:
                starts.append((cum, cum + nf))
                cum += nf
            ncalls = len(CALL_COLS)
            first_touch = {}
            for k, (c0, c1) in enumerate(starts):
                for c in range(c0 // N_SPARSE, (c1 - 1) // N_SPARSE + 1):
                    first_touch.setdefault(c, k)
            # prepare chunk c's dense matmuls 2 calls before first touch so
            # PE isn't stalled by them at the boundary
            prep_at = {}
            for c, k in first_touch.items():
                prep_at.setdefault(max(0, k - 2), []).append(c)

            def emit_gather(k):
                c0, c1 = starts[k]
                nf = c1 - c0
                g = gath.tile([P, 8, ROW], F32, tag="g")
                nc.gpsimd.dma_gather(
                    g[:, :nf, :],
                    vaug_t[:],
                    idx_ap(c0, c1),
                    nf * P,
                    nf * P,
                    ROW,
                    single_packet=True,
                    queue_num=k % N_QUEUES,
                )
                return g

            pending_epi = []   # (emit_at_call, chunk, ps)

            def process_call(k, g, eng=None):
                c0, c1 = starts[k]
                nf = c1 - c0
                j = 0
                while j < nf:
                    gcol = c0 + j
                    c = gcol // N_SPARSE
                    seg_end = min(nf, (c + 1) * N_SPARSE - c0)
                    ps = ensure_chunk(c)
                    is_last = (c0 + seg_end) == (c + 1) * N_SPARSE
                    cols = fold(g, j, seg_end, eng=eng)
                    accum(ps, cols, stop=is_last)
                    if is_last:
                        # defer the epilogue so its PSUM reads enter the
                        # in-order DVE/ACT queues after the stop retires
                        pending_epi.append((k + EPI_DEFER, c, ps))
                    j = seg_end
                while pending_epi and pending_epi[0][0] <= k:
                    _, c, ps = pending_epi.pop(0)
                    epilogue(c, ps)

            # call 0's gather first (gated only by idx piece 0), then the
            # remaining input loads, then call 0's processing
            g0 = emit_gather(0)
            load_idx_piece(1)
            load_idx_piece(2)
            nc.sync.dma_start(ident[:], ident_t[:])
            nc.sync.dma_start(combo[:], combo_t[:])
            for c in prep_at.get(0, []):
                ensure_chunk(c)
            process_call(0, g0)

            for k in range(1, ncalls):
                for c in prep_at.get(k, []):
                    ensure_chunk(c)
                g = emit_gather(k)
                process_call(k, g)
            for _, c, ps in pending_epi:
                epilogue(c, ps)

            nc.sync.dma_start(out_t[:], res[:])

    nc.compile()
    return nc


def prep_inputs(dense_inputs, sparse_inputs, w0, w, v):
    """Host-side shard/pack: build per-core in_maps."""
    dense = np.asarray(dense_inputs, np.float32)
    sparse = np.asarray(sparse_inputs)
    w0 = np.asarray(w0, np.float32)
    w = np.asarray(w, np.float32)
    v = np.asarray(v, np.float32)

    vaug = np.zeros((FEATURE_NUM, ROW), np.float32)
    # k-major v layout: col k*39+f = v[r, f, k]
    vaug[:, :VCOLS] = v.transpose(0, 2, 1).reshape(FEATURE_NUM, VCOLS)
    vaug[:, VCOLS] = w[:, 0]
    # fold w0 into sparse table 0 (each sample hits it exactly once)
    vaug[N_DENSE : N_DENSE + FEAT_PER_SPARSE, VCOLS] += w0[0]
    # V_s row field-sums for k=0..14 ride in the pad cols
    vaug[:, VCOLS + 1 : ROW] = v.sum(axis=1)[:, 0:15]

    offs = N_DENSE + FEAT_PER_SPARSE * np.arange(N_SPARSE, dtype=np.int64)
    gidx = (sparse.astype(np.int64) + offs[None, :]).astype(np.int16)  # [B, 26]

    in_maps = []
    for core in range(N_CORES):
        sl = slice(core * BC, (core + 1) * BC)
        idxc = gidx[sl]                                 # [512, 26]
        combo = np.zeros((N_DENSE, ROW + BC), np.float32)
        combo[:, 0:ROW] = vaug[0:N_DENSE]
        combo[:, ROW:] = dense[sl].T                    # [13, 512]

        # flat col order: gcol = chunk*26 + field; call layout per CALL_COLS
        buf = np.zeros((P, TOTCOLS * 8), np.int16)
        cum = 0
        for nf in CALL_COLS:
            n = nf * P
            cols = []
            for gcol in range(cum, cum + nf):
                c, f = divmod(gcol, N_SPARSE)
                cols.append(idxc[c * P : (c + 1) * P, f])   # [128]
            seg = np.stack(cols, 0).reshape(-1)             # [nf*128]
            wrapped = seg.reshape(n // 16, 16).T            # [16, n/16]
            buf[:, cum * 8 : (cum + nf) * 8] = np.tile(wrapped, (8, 1))
            cum += nf
        in_maps.append({"vaug": vaug, "idxs": buf, "combo13": combo,
                        "ident": np.eye(P, dtype=np.float32)})
    return in_maps


_NC_CACHE = None


def kernel(dense_inputs, sparse_inputs, w0, w, v):
    global _NC_CACHE
    if _NC_CACHE is None:
        _NC_CACHE = build_program()
    nc = _NC_CACHE
    in_maps = prep_inputs(dense_inputs, sparse_inputs, w0, w, v)
    res = run_bass_kernel_spmd(nc, in_maps, core_ids=list(range(N_CORES)))
    outs = []
    for r in res.results:
        o = r["out"]                                    # [128, 4]
        outs.append(np.ascontiguousarray(o.T).reshape(BC, 1))
    return np.concatenate(outs, axis=0).astype(np.float32)


# revision 54
# speedup vs baseline: 1.0435x; 1.0002x over previous
"""FFM layer (nn_FFM_Layer) Trainium2 Bass kernel.

Reference computation (B=4096, 13 dense fields, 26 sparse fields with vocab
1000 each, FIELD_NUM=39, K=16):

    idx        = sparse + offsets                      # [B, 26] global ids
    first      = w0 + dense @ w[:13] + sum_j w[idx]    # [B, 1]
    field_f    = einsum('bd,dfk', dense, v[:13]) + sum_j v[idx]   # [B,39,16]
    s          = field_f.sum(1)                        # [B, 16]
    second     = 0.5*(||s||^2 - sum_fk field_f^2)      # [B]
    out        = first + second[:, None]

Strategy (data-parallel over batch, 8 cores x 512 samples, no collectives):
  * Host packs an augmented table V_AUG [26013, 640] f32 (2560 B rows, the
    256B-multiple dma_gather granularity):
      cols [0:624]  = v in K-MAJOR layout (col k*39+f = v[r,f,k])
      col  624      = w[:, 0]  (+ w0 folded into sparse table 0 rows)
      cols [625:640]= V_s[r, 0:15] = per-row field-sums sum_f v[r,f,0:15]
    K-major + V_s pads make the s-vector nearly free: the fold tree and the
    dense matmul sum the pad cols along with everything else, so
    s[0:15] = psum[625:640] and s[15] = one contiguous 39-col reduce.
  * Each core dma_gathers its 512*26 = 13312 rows as ONE flat call stream
    (cols chunk-major: 4 sample-chunks x 26 fields = 104 cols) sized
    [1,1,2,4, 8x9, 4x5, 2,1,1]: small ramp calls because descriptors only
    start executing when a whole call's desc-gen finishes (early doorbells
    start the drain sooner), 8-col calls in the middle for serial Q7
    desc-gen throughput, and a fine-grained back half so the final
    data->fold->stop latency is small.  single_packet=True and queue_num
    round-robin over all 4 SWDGE queues measured fastest.
  * Startup facts (measured): the first DMAGatherAnt cannot execute before
    ~16.5us regardless of inputs (fixed SWDGE/Q7 wake-up after
    LIBRARY_RELOAD; even a zero-input warmup gather waits).  Input DMAs are
    ordered so only the 8-col idx piece gates call 0; the bulk idx pieces,
    ident and the combined [13, 640+512] vaug13+dense tensor are emitted
    after the first gather (DMA-completion waits are cumulative per lane,
    so anything emitted earlier would gate the stream).
  * Per call, DVE folds the gathered cols to one col (pairwise tree;
    tree-on-DVE beats offloading fold levels to PE, whose identity-matmul
    accumulation costs ~1.4us/col with weight reloads vs 0.74us/col adds)
    and PE accumulates it into the chunk's PSUM chain (512/128-col matmul
    split per PSUM bank) seeded by the dense [13,128]x[13,640] matmul,
    emitted 2 calls ahead of each chunk boundary.
  * Epilogue per chunk (emitted 2 calls AFTER the stop matmul so its PSUM
    reads never block the in-order DVE/ACT queues): ACT Square+accum_out
    over ps[0:624] -> sum field_f^2; ACT Identity copy of ps[624:640]
    (w-sum + s[0:15]); DVE 39-col reduce -> s[15]; snorm/diff tiny DVE ops;
    final ACT scale-0.5 + w-sum bias; one output DMA at the end.

Measured on HW (min over reps): 120.5-121.5 us vs 128.8 us baseline.
Breakdown: ~16.5 fixed SWDGE wake-up, ~88-92 desc-gen/drain (all 16 DMA
engines saturated, ~6 ns/row aggregate; Q7 desc-gen ~5-6 ns/row serial is
the pacer), ~10-14 fold/epilogue/output tail.  Host-side prefetch of the
first columns via bulk HWDGE DMA into the wake-up window measured neutral
(the transfer contends with the stream ramp) and was dropped.
"""

import sys

if "/opt/trn_rl_repo" not in sys.path:
    sys.path.insert(0, "/opt/trn_rl_repo")

import numpy as np

import concourse.bacc as bacc
import concourse.bass as bass
import concourse.tile as tile
from concourse import mybir
from concourse.bass_utils import run_bass_kernel_spmd

# Problem constants (hardcoded per harness contract)
B = 4096
N_DENSE = 13
N_SPARSE = 26
FEAT_PER_SPARSE = 1000
FIELD_NUM = 39
FEATURE_NUM = 26013
K = 16
N_CORES = 8
BC = B // N_CORES          # 512 samples per core
ROW = 640                  # 624 v (k-major) + 1 w + 15 V_s  (2560 B)
VCOLS = FIELD_NUM * K      # 624
P = 128
SCHUNKS = BC // P          # 4 sample chunks of 128 per core
TOTCOLS = SCHUNKS * N_SPARSE   # 104 flat gather cols per core

# flat call sizes (cols per dma_gather call).  Small ramp calls make the
# first SWDGE doorbells (and hence the drain) start early -- descriptors
# only begin executing when a whole call's desc-gen completes.  Big calls
# after that maximize serial desc-gen throughput (~5 ns/row + ~1us/call);
# small tail calls shorten the final data/fold latency.
# First NPRE cols are host-packed and loaded as one bulk HWDGE 2D DMA that
# rides the otherwise-dead SWDGE wake-up window; the SWDGE stream covers
# cols NPRE..104 (1024 fewer serial Q7 descriptors).
NPRE = 8
CALL_COLS = [1, 1, 2, 4] + [8] * 8 + [4] * 5 + [2, 1, 1]
assert sum(CALL_COLS) == TOTCOLS - NPRE
N_QUEUES = 4
# idx tensor split: first piece loads first so call-0 desc-gen isn't gated
# by the full 213KB idx transfer
IDX_SPLITS = [8, 24, TOTCOLS - NPRE - 32]   # cols per idx sub-tensor
EPI_DEFER = 2                               # calls to defer epilogue emission

F32 = mybir.dt.float32
I16 = mybir.dt.int16


def build_program():
    """Build + compile the single-core SPMD bass program."""
    nc = bacc.Bacc("TRN2", target_bir_lowering=False, debug=False,
                   num_swdge_queues=N_QUEUES)

    vaug_t = nc.dram_tensor("vaug", [FEATURE_NUM, ROW], F32, kind="ExternalInput")
    idxs_t = nc.dram_tensor("idxs", [P, TOTCOLS * 8], I16, kind="ExternalInput")
    combo_t = nc.dram_tensor("combo13", [N_DENSE, ROW + BC], F32,
                             kind="ExternalInput")
    ident_t = nc.dram_tensor("ident", [P, P], F32, kind="ExternalInput")
    pre_t = nc.dram_tensor("pre", [P, NPRE * ROW], F32, kind="ExternalInput")
    out_t = nc.dram_tensor("out", [P, SCHUNKS], F32, kind="ExternalOutput")

    with tile.TileContext(nc) as tc:
        with (
            tc.tile_pool(name="main", bufs=1) as main,
            tc.tile_pool(name="gath", bufs=6) as gath,
            tc.tile_pool(name="f4", bufs=3) as f4p,
            tc.tile_pool(name="f2", bufs=2) as f2p,
            tc.tile_pool(name="f1", bufs=2) as f1p,
            tc.tile_pool(name="sq", bufs=2) as sqp,
            tc.tile_pool(name="small", bufs=2) as small,
            tc.tile_pool(name="psum", bufs=4, space="PSUM") as psum,
        ):
            # idx piece 0 loads FIRST and alone: it is the only DMA gating
            # the ramp gather calls, whose desc-gen should start ~8us.  The
            # remaining input DMAs are emitted AFTER the ramp gathers so no
            # false ordering/semaphore dependency can delay the stream start.
            pre_sb = main.tile([P, NPRE, ROW], F32, tag="pre")

            idx_tiles = []     # (tile, first_col, end_col)
            col0 = NPRE
            for si, ncols in enumerate(IDX_SPLITS):
                t = main.tile([P, ncols * 8], I16, tag=f"idx{si}")
                idx_tiles.append((t, col0, col0 + ncols))
                col0 += ncols
            nc.sync.dma_start(
                idx_tiles[0][0][:],
                idxs_t[:, NPRE * 8 : (NPRE + IDX_SPLITS[0]) * 8],
            )

            def load_idx_piece(si):
                t, lo, hi = idx_tiles[si]
                nc.sync.dma_start(t[:], idxs_t[:, lo * 8 : hi * 8])

            def idx_ap(c0, c1):
                """SBUF idx AP covering gather cols [c0, c1)."""
                for t, lo, hi in idx_tiles:
                    if c0 >= lo and c1 <= hi:
                        return t[:, (c0 - lo) * 8 : (c1 - lo) * 8]
                raise AssertionError((c0, c1))

            ident = main.tile([P, P], F32)
            combo = main.tile([N_DENSE, ROW + BC], F32)
            vaug13 = combo[:, 0:ROW]

            res = main.tile([P, SCHUNKS], F32)

            ps_tiles = {}

            def ensure_chunk(c):
                """Seed chunk c's PSUM chain with the dense matmul."""
                if c in ps_tiles:
                    return ps_tiles[c]
                ps = psum.tile([P, ROW], F32, tag="ps")
                lhs_d = combo[:, ROW + c * P : ROW + (c + 1) * P]
                nc.tensor.matmul(out=ps[:, 0:512], lhsT=lhs_d,
                                 rhs=vaug13[:, 0:512], start=True, stop=False)
                nc.tensor.matmul(out=ps[:, 512:ROW], lhsT=lhs_d,
                                 rhs=vaug13[:, 512:ROW], start=True, stop=False)
                ps_tiles[c] = ps
                return ps

            def accum(ps, cols, stop):
                """PE identity-matmul accumulate [P, ROW] cols into ps."""
                for i, col_ap in enumerate(cols):
                    last = stop and i == len(cols) - 1
                    nc.tensor.matmul(out=ps[:, 0:512], lhsT=ident[:],
                                     rhs=col_ap[:, 0:512], start=False,
                                     stop=last)
                    nc.tensor.matmul(out=ps[:, 512:ROW], lhsT=ident[:],
                                     rhs=col_ap[:, 512:ROW], start=False,
                                     stop=last)

            def fold(g, j0, j1, eng=None):
                """DVE pairwise-tree fold g[:, j0:j1, :] -> one [P, ROW] col.

                Tree-on-DVE beats splitting the reduction with PE mid-stream
                (identity-matmul accumulation costs ~1.4us/col with weight
                reloads vs 0.74us/col DVE adds).  Segment sizes are
                {1,2,3,4,6,8} by construction.
                """
                ve = eng if eng is not None else nc.vector
                n = j1 - j0
                if n == 1:
                    return [g[:, j0, :]]
                if n == 2:
                    t1 = f1p.tile([P, ROW], F32, tag="f1")
                    ve.tensor_add(t1[:], g[:, j0, :], g[:, j0 + 1, :])
                    return [t1]
                if n == 3:
                    t2 = f2p.tile([P, 2, ROW], F32, tag="f2")
                    ve.tensor_add(t2[:, 0:1, :], g[:, j0 : j0 + 1, :],
                                  g[:, j0 + 1 : j0 + 2, :])
                    t1 = f1p.tile([P, ROW], F32, tag="f1")
                    ve.tensor_add(t1[:], t2[:, 0, :], g[:, j0 + 2, :])
                    return [t1]
                if n == 4:
                    t2 = f2p.tile([P, 2, ROW], F32, tag="f2")
                    ve.tensor_add(t2[:], g[:, j0 : j0 + 2, :],
                                  g[:, j0 + 2 : j0 + 4, :])
                    t1 = f1p.tile([P, ROW], F32, tag="f1")
                    ve.tensor_add(t1[:], t2[:, 0, :], t2[:, 1, :])
                    return [t1]
                if n == 6:
                    t4 = f4p.tile([P, 4, ROW], F32, tag="f4")
                    ve.tensor_add(t4[:, 0:3, :], g[:, j0 : j0 + 3, :],
                                  g[:, j0 + 3 : j0 + 6, :])
                    t2 = f2p.tile([P, 2, ROW], F32, tag="f2")
                    ve.tensor_add(t2[:, 0:1, :], t4[:, 0:1, :], t4[:, 1:2, :])
                    t1 = f1p.tile([P, ROW], F32, tag="f1")
                    ve.tensor_add(t1[:], t2[:, 0, :], t4[:, 2, :])
                    return [t1]
                assert n == 8, n
                t4 = f4p.tile([P, 4, ROW], F32, tag="f4")
                ve.tensor_add(t4[:], g[:, j0 : j0 + 4, :],
                              g[:, j0 + 4 : j0 + 8, :])
                t2 = f2p.tile([P, 2, ROW], F32, tag="f2")
                ve.tensor_add(t2[:], t4[:, 0:2, :], t4[:, 2:4, :])
                t1 = f1p.tile([P, ROW], F32, tag="f1")
                ve.tensor_add(t1[:], t2[:, 0, :], t2[:, 1, :])
                return [t1]

            def epilogue(c, ps):
                """FM identity epilogue for chunk c, reading PSUM directly."""
                # q = sum field_f^2  over the 624 v cols
                sqt = sqp.tile([P, VCOLS], F32, tag="sqt")
                q = small.tile([P, 1], F32, tag=f"q{c}")
                nc.scalar.activation(
                    sqt[:], ps[:, 0:VCOLS],
                    mybir.ActivationFunctionType.Square, accum_out=q[:],
                )
                # s[15] = contiguous 39-col reduce of the k=15 block
                st15 = small.tile([P, 1], F32, tag=f"st15{c}")
                nc.vector.tensor_reduce(
                    out=st15[:], in_=ps[:, 15 * FIELD_NUM : VCOLS],
                    op=mybir.AluOpType.add, axis=mybir.AxisListType.X,
                )
                # w-sum + s[0:15] copied out of PSUM once on ACT; the copy
                # also provides the SBUF bias source for the final combine
                wst = small.tile([P, 16], F32, tag=f"wst{c}")
                nc.scalar.activation(
                    wst[:], ps[:, VCOLS:ROW],
                    mybir.ActivationFunctionType.Identity,
                )
                # snorm = sum_{k<15} s_k^2 + s15^2
                s2 = small.tile([P, 15], F32, tag=f"s2{c}")
                sn = small.tile([P, 1], F32, tag=f"sn{c}")
                nc.scalar.activation(
                    s2[:], wst[:, 1:16],
                    mybir.ActivationFunctionType.Square, accum_out=sn[:],
                )
                s15sq = small.tile([P, 1], F32, tag=f"s15sq{c}")
                nc.vector.tensor_tensor(
                    out=s15sq[:], in0=st15[:], in1=st15[:],
                    op=mybir.AluOpType.mult,
                )
                nc.vector.tensor_tensor(
                    out=sn[:], in0=sn[:], in1=s15sq[:],
                    op=mybir.AluOpType.add,
                )
                diff = small.tile([P, 1], F32, tag=f"diff{c}")
                nc.vector.tensor_tensor(
                    out=diff[:], in0=sn[:], in1=q[:],
                    op=mybir.AluOpType.subtract,
                )
                # out = 0.5*diff + (w-sum incl. w0 and dense first-order)
                nc.scalar.activation(
                    res[:, c : c + 1], diff[:],
                    mybir.ActivationFunctionType.Identity,
                    bias=wst[:, 0:1], scale=0.5,
                )

            # precompute per-call col ranges and chunk prepare/epilogue sched
            starts = []
            cum = NPRE
            for nf in CALL_COLS:
                starts.append((cum, cum + nf))
                cum += nf
            ncalls = len(CALL_COLS)
            first_touch = {}
            for k, (c0, c1) in enumerate(starts):
                for c in range(c0 // N_SPARSE, (c1 - 1) // N_SPARSE + 1):
                    first_touch.setdefault(c, k)
            # prepare chunk c's dense matmuls 2 calls before first touch so
            # PE isn't stalled by them at the boundary
            prep_at = {}
            for c, k in first_touch.items():
                prep_at.setdefault(max(0, k - 2), []).append(c)

            def emit_gather(k):
                c0, c1 = starts[k]
                nf = c1 - c0
                g = gath.tile([P, 8, ROW], F32, tag="g")
                nc.gpsimd.dma_gather(
                    g[:, :nf, :],
                    vaug_t[:],
                    idx_ap(c0, c1),
                    nf * P,
                    nf * P,
                    ROW,
                    single_packet=True,
                    queue_num=k % N_QUEUES,
                )
                return g

            pending_epi = []   # (emit_at_call, chunk, ps)

            def process_call(k, g, eng=None):
                c0, c1 = starts[k]
                nf = c1 - c0
                j = 0
                while j < nf:
                    gcol = c0 + j
                    c = gcol // N_SPARSE
                    seg_end = min(nf, (c + 1) * N_SPARSE - c0)
                    ps = ensure_chunk(c)
                    is_last = (c0 + seg_end) == (c + 1) * N_SPARSE
                    cols = fold(g, j, seg_end, eng=eng)
                    accum(ps, cols, stop=is_last)
                    if is_last:
                        # defer the epilogue so its PSUM reads enter the
                        # in-order DVE/ACT queues after the stop retires
                        pending_epi.append((k + EPI_DEFER, c, ps))
                    j = seg_end
                while pending_epi and pending_epi[0][0] <= k:
                    _, c, ps = pending_epi.pop(0)
                    epilogue(c, ps)

            # call 0's gather first (gated only by idx piece 0), then the
            # remaining input loads, then call 0's processing
            g0 = emit_gather(0)
            # bulk prefetch AFTER the first gather: DMA-completion waits are
            # cumulative per lane, so emitting it earlier gates the stream
            nc.sync.dma_start(pre_sb[:], pre_t[:])
            load_idx_piece(1)
            load_idx_piece(2)
            nc.sync.dma_start(ident[:], ident_t[:])
            nc.sync.dma_start(combo[:], combo_t[:])
            # prefetched cols 0..NPRE (chunk 0) fold while SWDGE wakes up
            ps0 = ensure_chunk(0)
            cols = fold(pre_sb, 0, NPRE)
            accum(ps0, cols, stop=False)
            for c in prep_at.get(0, []):
                ensure_chunk(c)
            process_call(0, g0)

            for k in range(1, ncalls):
                for c in prep_at.get(k, []):
                    ensure_chunk(c)
                g = emit_gather(k)
                process_call(k, g)
            for _, c, ps in pending_epi:
                epilogue(c, ps)

            nc.sync.dma_start(out_t[:], res[:])

    nc.compile()
    return nc


def prep_inputs(dense_inputs, sparse_inputs, w0, w, v):
    """Host-side shard/pack: build per-core in_maps."""
    dense = np.asarray(dense_inputs, np.float32)
    sparse = np.asarray(sparse_inputs)
    w0 = np.asarray(w0, np.float32)
    w = np.asarray(w, np.float32)
    v = np.asarray(v, np.float32)

    vaug = np.zeros((FEATURE_NUM, ROW), np.float32)
    # k-major v layout: col k*39+f = v[r, f, k]
    vaug[:, :VCOLS] = v.transpose(0, 2, 1).reshape(FEATURE_NUM, VCOLS)
    vaug[:, VCOLS] = w[:, 0]
    # fold w0 into sparse table 0 (each sample hits it exactly once)
    vaug[N_DENSE : N_DENSE + FEAT_PER_SPARSE, VCOLS] += w0[0]
    # V_s row field-sums for k=0..14 ride in the pad cols
    vaug[:, VCOLS + 1 : ROW] = v.sum(axis=1)[:, 0:15]

    offs = N_DENSE + FEAT_PER_SPARSE * np.arange(N_SPARSE, dtype=np.int64)
    gidx = (sparse.astype(np.int64) + offs[None, :]).astype(np.int16)  # [B, 26]

    in_maps = []
    for core in range(N_CORES):
        sl = slice(core * BC, (core + 1) * BC)
        idxc = gidx[sl]                                 # [512, 26]
        combo = np.zeros((N_DENSE, ROW + BC), np.float32)
        combo[:, 0:ROW] = vaug[0:N_DENSE]
        combo[:, ROW:] = dense[sl].T                    # [13, 512]

        # host-gathered prefetch block: cols 0..NPRE of chunk 0, packed so
        # partition p holds sample p's NPRE rows
        rows = idxc[0:P, 0:NPRE].astype(np.int64)       # [128, NPRE]
        pre = vaug[rows.T.reshape(-1)]                  # [NPRE*128, 640]
        pre = np.ascontiguousarray(
            pre.reshape(NPRE, P, ROW).transpose(1, 0, 2)
        ).reshape(P, NPRE * ROW)

        # flat col order: gcol = chunk*26 + field; call layout per CALL_COLS
        buf = np.zeros((P, TOTCOLS * 8), np.int16)
        cum = NPRE
        for nf in CALL_COLS:
            n = nf * P
            cols = []
            for gcol in range(cum, cum + nf):
                c, f = divmod(gcol, N_SPARSE)
                cols.append(idxc[c * P : (c + 1) * P, f])   # [128]
            seg = np.stack(cols, 0).reshape(-1)             # [nf*128]
            wrapped = seg.reshape(n // 16, 16).T            # [16, n/16]
            buf[:, cum * 8 : (cum + nf) * 8] = np.tile(wrapped, (8, 1))
            cum += nf
        in_maps.append({"vaug": vaug, "idxs": buf, "combo13": combo,
                        "pre": pre, "ident": np.eye(P, dtype=np.float32)})
    return in_maps


_NC_CACHE = None


def kernel(dense_inputs, sparse_inputs, w0, w, v):
    global _NC_CACHE
    if _NC_CACHE is None:
        _NC_CACHE = build_program()
    nc = _NC_CACHE
    in_maps = prep_inputs(dense_inputs, sparse_inputs, w0, w, v)
    res = run_bass_kernel_spmd(nc, in_maps, core_ids=list(range(N_CORES)))
    outs = []
    for r in res.results:
        o = r["out"]                                    # [128, 4]
        outs.append(np.ascontiguousarray(o.T).reshape(BC, 1))
    return np.concatenate(outs, axis=0).astype(np.float32)


# revision 55
# speedup vs baseline: 1.0495x; 1.0058x over previous
"""FFM layer (nn_FFM_Layer) Trainium2 Bass kernel.

Reference computation (B=4096, 13 dense fields, 26 sparse fields with vocab
1000 each, FIELD_NUM=39, K=16):

    idx        = sparse + offsets                      # [B, 26] global ids
    first      = w0 + dense @ w[:13] + sum_j w[idx]    # [B, 1]
    field_f    = einsum('bd,dfk', dense, v[:13]) + sum_j v[idx]   # [B,39,16]
    s          = field_f.sum(1)                        # [B, 16]
    second     = 0.5*(||s||^2 - sum_fk field_f^2)      # [B]
    out        = first + second[:, None]

Strategy (data-parallel over batch, 8 cores x 512 samples, no collectives):
  * Host packs an augmented table V_AUG [26013, 640] f32 (2560 B rows, the
    256B-multiple dma_gather granularity):
      cols [0:624]  = v in K-MAJOR layout (col k*39+f = v[r,f,k])
      col  624      = w[:, 0]  (+ w0 folded into sparse table 0 rows)
      cols [625:640]= V_s[r, 0:15] = per-row field-sums sum_f v[r,f,0:15]
    K-major + V_s pads make the s-vector nearly free: the fold tree and the
    dense matmul sum the pad cols along with everything else, so
    s[0:15] = psum[625:640] and s[15] = one contiguous 39-col reduce.
  * Each core dma_gathers its 512*26 = 13312 rows as ONE flat call stream
    (cols chunk-major: 4 sample-chunks x 26 fields = 104 cols) sized
    [1,1,2,4, 8x9, 4x5, 2,1,1]: small ramp calls because descriptors only
    start executing when a whole call's desc-gen finishes (early doorbells
    start the drain sooner), 8-col calls in the middle for serial Q7
    desc-gen throughput, and a fine-grained back half so the final
    data->fold->stop latency is small.  single_packet=True and queue_num
    round-robin over all 4 SWDGE queues measured fastest.
  * Startup facts (measured): the first DMAGatherAnt cannot execute before
    ~16.5us regardless of inputs (fixed SWDGE/Q7 wake-up after
    LIBRARY_RELOAD; even a zero-input warmup gather waits).  Input DMAs are
    ordered so only the 8-col idx piece gates call 0; the bulk idx pieces,
    ident and the combined [13, 640+512] vaug13+dense tensor are emitted
    after the first gather (DMA-completion waits are cumulative per lane,
    so anything emitted earlier would gate the stream).
  * Per call, DVE folds the gathered cols to one col (pairwise tree;
    tree-on-DVE beats offloading fold levels to PE, whose identity-matmul
    accumulation costs ~1.4us/col with weight reloads vs 0.74us/col adds)
    and PE accumulates it into the chunk's PSUM chain (512/128-col matmul
    split per PSUM bank) seeded by the dense [13,128]x[13,640] matmul,
    emitted 2 calls ahead of each chunk boundary.
  * Epilogue per chunk (emitted 2 calls AFTER the stop matmul so its PSUM
    reads never block the in-order DVE/ACT queues): ACT Square+accum_out
    over ps[0:624] -> sum field_f^2; ACT Identity copy of ps[624:640]
    (w-sum + s[0:15]); DVE 39-col reduce -> s[15]; snorm/diff tiny DVE ops;
    final ACT scale-0.5 + w-sum bias; one output DMA at the end.

Measured on HW (min over reps): 120.5-121.5 us vs 128.8 us baseline.
Breakdown: ~16.5 fixed SWDGE wake-up, ~88-92 desc-gen/drain (all 16 DMA
engines saturated, ~6 ns/row aggregate; Q7 desc-gen ~5-6 ns/row serial is
the pacer), ~10-14 fold/epilogue/output tail.  Host-side prefetch of the
first columns via bulk HWDGE DMA into the wake-up window measured neutral
(the transfer contends with the stream ramp) and was dropped.
"""

import sys

if "/opt/trn_rl_repo" not in sys.path:
    sys.path.insert(0, "/opt/trn_rl_repo")

import numpy as np

import concourse.bacc as bacc
import concourse.bass as bass
import concourse.tile as tile
from concourse import mybir
from concourse.bass_utils import run_bass_kernel_spmd

# Problem constants (hardcoded per harness contract)
B = 4096
N_DENSE = 13
N_SPARSE = 26
FEAT_PER_SPARSE = 1000
FIELD_NUM = 39
FEATURE_NUM = 26013
K = 16
N_CORES = 8
BC = B // N_CORES          # 512 samples per core
ROW = 640                  # 624 v (k-major) + 1 w + 15 V_s  (2560 B)
VCOLS = FIELD_NUM * K      # 624
P = 128
SCHUNKS = BC // P          # 4 sample chunks of 128 per core
TOTCOLS = SCHUNKS * N_SPARSE   # 104 flat gather cols per core

# flat call sizes (cols per dma_gather call).  Small ramp calls make the
# first SWDGE doorbells (and hence the drain) start early -- descriptors
# only begin executing when a whole call's desc-gen completes.  Big calls
# after that maximize serial desc-gen throughput (~5 ns/row + ~1us/call);
# small tail calls shorten the final data/fold latency.
CALL_COLS = [1, 1, 2, 4] + [8] * 9 + [4] * 5 + [2, 1, 1]
assert sum(CALL_COLS) == TOTCOLS
N_QUEUES = 4
# idx tensor split: first piece loads first so call-0 desc-gen isn't gated
# by the full 213KB idx transfer
IDX_SPLITS = [8, 24, TOTCOLS - 32]          # cols per idx sub-tensor
EPI_DEFER = 2                               # calls to defer epilogue emission

F32 = mybir.dt.float32
I16 = mybir.dt.int16


def build_program():
    """Build + compile the single-core SPMD bass program."""
    nc = bacc.Bacc("TRN2", target_bir_lowering=False, debug=False,
                   num_swdge_queues=N_QUEUES)

    vaug_t = nc.dram_tensor("vaug", [FEATURE_NUM, ROW], F32, kind="ExternalInput")
    idxs_t = nc.dram_tensor("idxs", [P, TOTCOLS * 8], I16, kind="ExternalInput")
    combo_t = nc.dram_tensor("combo13", [N_DENSE, ROW + BC], F32,
                             kind="ExternalInput")
    ident_t = nc.dram_tensor("ident", [P, P], F32, kind="ExternalInput")
    out_t = nc.dram_tensor("out", [P, SCHUNKS], F32, kind="ExternalOutput")

    with tile.TileContext(nc) as tc:
        with (
            tc.tile_pool(name="main", bufs=1) as main,
            tc.tile_pool(name="gath", bufs=7) as gath,
            tc.tile_pool(name="f4", bufs=3) as f4p,
            tc.tile_pool(name="f2", bufs=2) as f2p,
            tc.tile_pool(name="f1", bufs=2) as f1p,
            tc.tile_pool(name="sq", bufs=2) as sqp,
            tc.tile_pool(name="small", bufs=2) as small,
            tc.tile_pool(name="psum", bufs=4, space="PSUM") as psum,
        ):
            # idx piece 0 loads FIRST and alone: it is the only DMA gating
            # the ramp gather calls, whose desc-gen should start ~8us.  The
            # remaining input DMAs are emitted AFTER the ramp gathers so no
            # false ordering/semaphore dependency can delay the stream start.
            idx_tiles = []     # (tile, first_col, end_col)
            col0 = 0
            for si, ncols in enumerate(IDX_SPLITS):
                t = main.tile([P, ncols * 8], I16, tag=f"idx{si}")
                idx_tiles.append((t, col0, col0 + ncols))
                col0 += ncols
            nc.sync.dma_start(idx_tiles[0][0][:], idxs_t[:, 0 : IDX_SPLITS[0] * 8])

            def load_idx_piece(si):
                t, lo, hi = idx_tiles[si]
                nc.sync.dma_start(t[:], idxs_t[:, lo * 8 : hi * 8])

            def idx_ap(c0, c1):
                """SBUF idx AP covering gather cols [c0, c1)."""
                for t, lo, hi in idx_tiles:
                    if c0 >= lo and c1 <= hi:
                        return t[:, (c0 - lo) * 8 : (c1 - lo) * 8]
                raise AssertionError((c0, c1))

            ident = main.tile([P, P], F32)
            combo = main.tile([N_DENSE, ROW + BC], F32)
            vaug13 = combo[:, 0:ROW]

            res = main.tile([P, SCHUNKS], F32)

            ps_tiles = {}

            def ensure_chunk(c):
                """Seed chunk c's PSUM chain with the dense matmul."""
                if c in ps_tiles:
                    return ps_tiles[c]
                ps = psum.tile([P, ROW], F32, tag="ps")
                lhs_d = combo[:, ROW + c * P : ROW + (c + 1) * P]
                nc.tensor.matmul(out=ps[:, 0:512], lhsT=lhs_d,
                                 rhs=vaug13[:, 0:512], start=True, stop=False)
                nc.tensor.matmul(out=ps[:, 512:ROW], lhsT=lhs_d,
                                 rhs=vaug13[:, 512:ROW], start=True, stop=False)
                ps_tiles[c] = ps
                return ps

            def accum(ps, cols, stop):
                """PE identity-matmul accumulate [P, ROW] cols into ps."""
                for i, col_ap in enumerate(cols):
                    last = stop and i == len(cols) - 1
                    nc.tensor.matmul(out=ps[:, 0:512], lhsT=ident[:],
                                     rhs=col_ap[:, 0:512], start=False,
                                     stop=last)
                    nc.tensor.matmul(out=ps[:, 512:ROW], lhsT=ident[:],
                                     rhs=col_ap[:, 512:ROW], start=False,
                                     stop=last)

            def fold(g, j0, j1, eng=None):
                """DVE pairwise-tree fold g[:, j0:j1, :] -> one [P, ROW] col.

                Tree-on-DVE beats splitting the reduction with PE mid-stream
                (identity-matmul accumulation costs ~1.4us/col with weight
                reloads vs 0.74us/col DVE adds).  Segment sizes are
                {1,2,3,4,6,8} by construction.
                """
                ve = eng if eng is not None else nc.vector
                n = j1 - j0
                if n == 1:
                    return [g[:, j0, :]]
                if n == 2:
                    t1 = f1p.tile([P, ROW], F32, tag="f1")
                    ve.tensor_add(t1[:], g[:, j0, :], g[:, j0 + 1, :])
                    return [t1]
                if n == 3:
                    t2 = f2p.tile([P, 2, ROW], F32, tag="f2")
                    ve.tensor_add(t2[:, 0:1, :], g[:, j0 : j0 + 1, :],
                                  g[:, j0 + 1 : j0 + 2, :])
                    t1 = f1p.tile([P, ROW], F32, tag="f1")
                    ve.tensor_add(t1[:], t2[:, 0, :], g[:, j0 + 2, :])
                    return [t1]
                if n == 4:
                    t2 = f2p.tile([P, 2, ROW], F32, tag="f2")
                    ve.tensor_add(t2[:], g[:, j0 : j0 + 2, :],
                                  g[:, j0 + 2 : j0 + 4, :])
                    t1 = f1p.tile([P, ROW], F32, tag="f1")
                    ve.tensor_add(t1[:], t2[:, 0, :], t2[:, 1, :])
                    return [t1]
                if n == 6:
                    t4 = f4p.tile([P, 4, ROW], F32, tag="f4")
                    ve.tensor_add(t4[:, 0:3, :], g[:, j0 : j0 + 3, :],
                                  g[:, j0 + 3 : j0 + 6, :])
                    t2 = f2p.tile([P, 2, ROW], F32, tag="f2")
                    ve.tensor_add(t2[:, 0:1, :], t4[:, 0:1, :], t4[:, 1:2, :])
                    t1 = f1p.tile([P, ROW], F32, tag="f1")
                    ve.tensor_add(t1[:], t2[:, 0, :], t4[:, 2, :])
                    return [t1]
                assert n == 8, n
                t4 = f4p.tile([P, 4, ROW], F32, tag="f4")
                ve.tensor_add(t4[:], g[:, j0 : j0 + 4, :],
                              g[:, j0 + 4 : j0 + 8, :])
                t2 = f2p.tile([P, 2, ROW], F32, tag="f2")
                ve.tensor_add(t2[:], t4[:, 0:2, :], t4[:, 2:4, :])
                t1 = f1p.tile([P, ROW], F32, tag="f1")
                ve.tensor_add(t1[:], t2[:, 0, :], t2[:, 1, :])
                return [t1]

            def epilogue(c, ps):
                """FM identity epilogue for chunk c, reading PSUM directly."""
                # q = sum field_f^2  over the 624 v cols
                sqt = sqp.tile([P, VCOLS], F32, tag="sqt")
                q = small.tile([P, 1], F32, tag=f"q{c}")
                nc.scalar.activation(
                    sqt[:], ps[:, 0:VCOLS],
                    mybir.ActivationFunctionType.Square, accum_out=q[:],
                )
                # s[15] = contiguous 39-col reduce of the k=15 block
                st15 = small.tile([P, 1], F32, tag=f"st15{c}")
                nc.vector.tensor_reduce(
                    out=st15[:], in_=ps[:, 15 * FIELD_NUM : VCOLS],
                    op=mybir.AluOpType.add, axis=mybir.AxisListType.X,
                )
                # w-sum + s[0:15] copied out of PSUM once on ACT; the copy
                # also provides the SBUF bias source for the final combine
                wst = small.tile([P, 16], F32, tag=f"wst{c}")
                nc.scalar.activation(
                    wst[:], ps[:, VCOLS:ROW],
                    mybir.ActivationFunctionType.Identity,
                )
                # snorm = sum_{k<15} s_k^2 + s15^2
                s2 = small.tile([P, 15], F32, tag=f"s2{c}")
                sn = small.tile([P, 1], F32, tag=f"sn{c}")
                nc.scalar.activation(
                    s2[:], wst[:, 1:16],
                    mybir.ActivationFunctionType.Square, accum_out=sn[:],
                )
                s15sq = small.tile([P, 1], F32, tag=f"s15sq{c}")
                nc.vector.tensor_tensor(
                    out=s15sq[:], in0=st15[:], in1=st15[:],
                    op=mybir.AluOpType.mult,
                )
                nc.vector.tensor_tensor(
                    out=sn[:], in0=sn[:], in1=s15sq[:],
                    op=mybir.AluOpType.add,
                )
                diff = small.tile([P, 1], F32, tag=f"diff{c}")
                nc.vector.tensor_tensor(
                    out=diff[:], in0=sn[:], in1=q[:],
                    op=mybir.AluOpType.subtract,
                )
                # out = 0.5*diff + (w-sum incl. w0 and dense first-order)
                nc.scalar.activation(
                    res[:, c : c + 1], diff[:],
                    mybir.ActivationFunctionType.Identity,
                    bias=wst[:, 0:1], scale=0.5,
                )

            # precompute per-call col ranges and chunk prepare/epilogue sched
            starts = []
            cum = 0
            for nf in CALL_COLS:
                starts.append((cum, cum + nf))
                cum += nf
            ncalls = len(CALL_COLS)
            first_touch = {}
            for k, (c0, c1) in enumerate(starts):
                for c in range(c0 // N_SPARSE, (c1 - 1) // N_SPARSE + 1):
                    first_touch.setdefault(c, k)
            # prepare chunk c's dense matmuls 2 calls before first touch so
            # PE isn't stalled by them at the boundary
            prep_at = {}
            for c, k in first_touch.items():
                prep_at.setdefault(max(0, k - 2), []).append(c)

            def emit_gather(k):
                c0, c1 = starts[k]
                nf = c1 - c0
                g = gath.tile([P, 8, ROW], F32, tag="g")
                nc.gpsimd.dma_gather(
                    g[:, :nf, :],
                    vaug_t[:],
                    idx_ap(c0, c1),
                    nf * P,
                    nf * P,
                    ROW,
                    single_packet=True,
                    queue_num=k % N_QUEUES,
                )
                return g

            pending_epi = []   # (emit_at_call, chunk, ps)

            def process_call(k, g, eng=None):
                c0, c1 = starts[k]
                nf = c1 - c0
                j = 0
                while j < nf:
                    gcol = c0 + j
                    c = gcol // N_SPARSE
                    seg_end = min(nf, (c + 1) * N_SPARSE - c0)
                    ps = ensure_chunk(c)
                    is_last = (c0 + seg_end) == (c + 1) * N_SPARSE
                    cols = fold(g, j, seg_end, eng=eng)
                    accum(ps, cols, stop=is_last)
                    if is_last:
                        # defer the epilogue so its PSUM reads enter the
                        # in-order DVE/ACT queues after the stop retires
                        pending_epi.append((k + EPI_DEFER, c, ps))
                    j = seg_end
                while pending_epi and pending_epi[0][0] <= k:
                    _, c, ps = pending_epi.pop(0)
                    epilogue(c, ps)

            # call 0's gather first (gated only by idx piece 0), then the
            # remaining input loads, then call 0's processing
            g0 = emit_gather(0)
            load_idx_piece(1)
            load_idx_piece(2)
            nc.sync.dma_start(ident[:], ident_t[:])
            nc.sync.dma_start(combo[:], combo_t[:])
            for c in prep_at.get(0, []):
                ensure_chunk(c)
            process_call(0, g0)

            for k in range(1, ncalls):
                for c in prep_at.get(k, []):
                    ensure_chunk(c)
                g = emit_gather(k)
                process_call(k, g)
            for _, c, ps in pending_epi:
                epilogue(c, ps)

            nc.sync.dma_start(out_t[:], res[:])

    nc.compile()
    return nc


def prep_inputs(dense_inputs, sparse_inputs, w0, w, v):
    """Host-side shard/pack: build per-core in_maps."""
    dense = np.asarray(dense_inputs, np.float32)
    sparse = np.asarray(sparse_inputs)
    w0 = np.asarray(w0, np.float32)
    w = np.asarray(w, np.float32)
    v = np.asarray(v, np.float32)

    vaug = np.zeros((FEATURE_NUM, ROW), np.float32)
    # k-major v layout: col k*39+f = v[r, f, k]
    vaug[:, :VCOLS] = v.transpose(0, 2, 1).reshape(FEATURE_NUM, VCOLS)
    vaug[:, VCOLS] = w[:, 0]
    # fold w0 into sparse table 0 (each sample hits it exactly once)
    vaug[N_DENSE : N_DENSE + FEAT_PER_SPARSE, VCOLS] += w0[0]
    # V_s row field-sums for k=0..14 ride in the pad cols
    vaug[:, VCOLS + 1 : ROW] = v.sum(axis=1)[:, 0:15]

    offs = N_DENSE + FEAT_PER_SPARSE * np.arange(N_SPARSE, dtype=np.int64)
    gidx = (sparse.astype(np.int64) + offs[None, :]).astype(np.int16)  # [B, 26]

    in_maps = []
    for core in range(N_CORES):
        sl = slice(core * BC, (core + 1) * BC)
        idxc = gidx[sl]                                 # [512, 26]
        combo = np.zeros((N_DENSE, ROW + BC), np.float32)
        combo[:, 0:ROW] = vaug[0:N_DENSE]
        combo[:, ROW:] = dense[sl].T                    # [13, 512]

        # flat col order: gcol = chunk*26 + field; call layout per CALL_COLS
        buf = np.zeros((P, TOTCOLS * 8), np.int16)
        cum = 0
        for nf in CALL_COLS:
            n = nf * P
            cols = []
            for gcol in range(cum, cum + nf):
                c, f = divmod(gcol, N_SPARSE)
                cols.append(idxc[c * P : (c + 1) * P, f])   # [128]
            seg = np.stack(cols, 0).reshape(-1)             # [nf*128]
            wrapped = seg.reshape(n // 16, 16).T            # [16, n/16]
            buf[:, cum * 8 : (cum + nf) * 8] = np.tile(wrapped, (8, 1))
            cum += nf
        in_maps.append({"vaug": vaug, "idxs": buf, "combo13": combo,
                        "ident": np.eye(P, dtype=np.float32)})
    return in_maps


_NC_CACHE = None


def kernel(dense_inputs, sparse_inputs, w0, w, v):
    global _NC_CACHE
    if _NC_CACHE is None:
        _NC_CACHE = build_program()
    nc = _NC_CACHE
    in_maps = prep_inputs(dense_inputs, sparse_inputs, w0, w, v)
    res = run_bass_kernel_spmd(nc, in_maps, core_ids=list(range(N_CORES)))
    outs = []
    for r in res.results:
        o = r["out"]                                    # [128, 4]
        outs.append(np.ascontiguousarray(o.T).reshape(BC, 1))
    return np.concatenate(outs, axis=0).astype(np.float32)
